# revision 77
# baseline (speedup 1.0000x reference)
"""DeformGAT (4-layer) Trainium2 kernel — 8 NeuronCores SPMD.

Sharding: nodes in 8 contiguous blocks of 1250 (padded to 1280); edges are
assigned to their dst node's core (edges pre-sorted by dst on host). Weights
replicated. Per layer each core gathers src rows of the replicated bf16
feature table (dma_gather), computes per-edge softmax with merged one-hot
matmuls (dst-score broadcast / segment-sum / reciprocal broadcast each as a
single free-48 matmul per slab), aggregates with bf16 scatter matmuls,
applies the per-head output transform (head-mean, bias row and coordinate
displacement folded into an augmented weight matrix), then AllGathers its
produced rows in 5 chunks per stage (overlapped with compute) so every core
again holds the full table.
"""
import numpy as np
import ml_dtypes
from contextlib import ExitStack

import concourse.bacc as bacc
import concourse.bass as bass
import concourse.tile as tile
import concourse.mybir as mybir
from concourse import library_config
from concourse.bass_utils import run_bass_kernel_spmd

F32 = mybir.dt.float32
BF16 = mybir.dt.bfloat16
I16 = mybir.dt.int16
AF = mybir.ActivationFunctionType
ALU = mybir.AluOpType
BF = ml_dtypes.bfloat16

NCORES = 8
N = 10000
E = 60000
H = 6
NL = 1250          # real nodes per core
NLP = 1280         # padded nodes per core
NBLK = 80          # dst blocks of 16 per core
NSLAB = 10         # slabs of 128 dst nodes (8 blocks)
CAP = 128          # edge capacity per block (= chunk)
NCHUNK = 5         # AllGather chunks per stage
CHROWS = NLP // NCHUNK   # rows per AG chunk per core (256)

# GAT layer dims (din, C). Stage s (2..5) runs GAT s-1.
GAT_DIMS = [(256, 508), (512, 250), (256, 120), (128, 20)]
FDIM = [256, 512, 256, 128]          # feat_s dim produced by stage s
ROWB = [384, 640, 384, 128]          # bf16 table row elems (stage-4 table is
                                     # compact: [c3(2), ss4(6), pad] — stage 5
                                     # only needs src coords + src scores)

SELU_L = 1.0507009873554805
SELU_A = 1.6732632423543772
LA = SELU_L * SELU_A

AG_MODE = "shared"   # "shared": 1 full-table Shared-output AG per stage
                     # "chunked": NCHUNK Local AGs per stage
GATHER_MODE = "indirect"    # "dma_gather" | "indirect" | "hoisted"
                           # hoisted: slabs 0-3 desc-prepped during the prior
                           # stage (SWDGE queues 0-3, fired at stage start);
                           # slabs 4-9 in-loop indirect DMA, 2-slab lookahead


def _build_nc():
    nc = bacc.Bacc("TRN2", target_bir_lowering=False, debug=False,
                   num_devices=NCORES, num_swdge_queues=4,
                   dynamic_dma_scratch_size=32768)
    # ---------------- inputs ----------------
    inp = {}
    inp["dataT"] = nc.dram_tensor("dataT", [16, NLP], BF16, kind="ExternalInput")
    inp["coords_loc"] = nc.dram_tensor("coords_loc", [NLP, 2], F32, kind="ExternalInput")
    inp["cfac"] = nc.dram_tensor("cfac", [NLP, 1], F32, kind="ExternalInput")
    inp["srcidx"] = nc.dram_tensor("srcidx", [128, NBLK * 8], I16, kind="ExternalInput")
    inp["sidx32"] = nc.dram_tensor("sidx32", [128, NSLAB * 8], mybir.dt.int32,
                                   kind="ExternalInput")
    inp["p0"] = nc.dram_tensor("p0", [128, NBLK * 16], F32, kind="ExternalInput")
    inp["p0stk"] = nc.dram_tensor("p0stk", [128, NSLAB * 128], F32, kind="ExternalInput")
    inp["p0rep"] = nc.dram_tensor("p0rep", [128, NBLK * 96], BF16, kind="ExternalInput")
    inp["diagmask"] = nc.dram_tensor("diagmask", [128, 48], F32, kind="ExternalInput")
    inp["ident"] = nc.dram_tensor("ident", [128, 128], BF16, kind="ExternalInput")
    inp["identf"] = nc.dram_tensor("identf", [128, 128], F32, kind="ExternalInput")
    inp["linW"] = nc.dram_tensor("linW", [16, 254], BF16, kind="ExternalInput")
    inp["bias1row"] = nc.dram_tensor("bias1row", [1, 254], BF16, kind="ExternalInput")
    inp["onesrow"] = nc.dram_tensor("onesrow", [1, 128], BF16, kind="ExternalInput")
    for i in range(1, 5):
        din, C = GAT_DIMS[i - 1]
        kt = din // 128
        CP = 2 if i == 4 else C + 2
        inp[f"wp{i}"] = nc.dram_tensor(f"wp{i}", [128, kt * 6 * CP], BF16, kind="ExternalInput")
        if i < 4:
            inp[f"biasrow{i}"] = nc.dram_tensor(f"biasrow{i}", [1, CP], BF16, kind="ExternalInput")
        ktf = FDIM[i - 1] // 128
        inp[f"wsc{i}"] = nc.dram_tensor(f"wsc{i}", [128, ktf * 12], BF16, kind="ExternalInput")
    out_t = nc.dram_tensor("out", [NLP, 2], F32, kind="ExternalOutput")

    rg = [list(range(NCORES))]

    with tile.TileContext(nc) as tc, ExitStack() as ctx:
        persist = ctx.enter_context(tc.tile_pool(name="persist", bufs=1))
        dram = ctx.enter_context(tc.tile_pool(name="dram", bufs=1, space="DRAM"))
        fg_pool = ctx.enter_context(tc.tile_pool(name="fg", bufs=3))
        gt_pool = ctx.enter_context(tc.tile_pool(name="gt", bufs=2))
        m_pool = ctx.enter_context(tc.tile_pool(name="m", bufs=2))
        e_pool = ctx.enter_context(tc.tile_pool(name="ep", bufs=2))
        fn_pool = ctx.enter_context(tc.tile_pool(name="fn", bufs=2))
        fnt_pool = ctx.enter_context(tc.tile_pool(name="fnt", bufs=2))
        wp_pool = ctx.enter_context(tc.tile_pool(name="wp", bufs=1))
        small = ctx.enter_context(tc.tile_pool(name="small", bufs=3))
        ps_gt = ctx.enter_context(tc.tile_pool(name="psgt", bufs=2, space="PSUM"))
        ps_f = ctx.enter_context(tc.tile_pool(name="psf", bufs=2, space="PSUM"))
        ps_sm = ctx.enter_context(tc.tile_pool(name="pssm", bufs=2, space="PSUM"))

        # ------------- resident loads -------------
        srcidx_sb = persist.tile([128, NBLK * 8], I16)
        nc.sync.dma_start(srcidx_sb[:], inp["srcidx"][:])
        sidx32_sb = persist.tile([128, NSLAB * 8], mybir.dt.int32)
        nc.sync.dma_start(sidx32_sb[:], inp["sidx32"][:])
        p0_sb = persist.tile([128, NBLK * 16], F32)
        nc.sync.dma_start(p0_sb[:], inp["p0"][:])
        p0stk_sb = persist.tile([128, NSLAB * 128], F32)
        nc.sync.dma_start(p0stk_sb[:], inp["p0stk"][:])
        p0rep_sb = persist.tile([128, NBLK * 96], BF16)
        nc.sync.dma_start(p0rep_sb[:], inp["p0rep"][:])
        diagmask_sb = persist.tile([128, 48], F32)
        nc.sync.dma_start(diagmask_sb[:], inp["diagmask"][:])
        ident_sb = persist.tile([128, 128], BF16)
        nc.sync.dma_start(ident_sb[:], inp["ident"][:])
        identf_sb = persist.tile([128, 128], F32)
        nc.sync.dma_start(identf_sb[:], inp["identf"][:])
        tabT_sb = persist.tile([16, NCORES * NLP], F32)
        dataT_sb = persist.tile([16, NLP], BF16)
        nc.sync.dma_start(dataT_sb[:], inp["dataT"][:])
        linW_sb = persist.tile([16, 254], BF16)
        nc.sync.dma_start(linW_sb[:], inp["linW"][:])
        bias1row_sb = persist.tile([1, 254], BF16)
        nc.sync.dma_start(bias1row_sb[:], inp["bias1row"][:])
        onesrow_sb = persist.tile([1, 128], BF16)
        nc.sync.dma_start(onesrow_sb[:], inp["onesrow"][:])
        cloc_sb = persist.tile([128, NSLAB, 2], F32)
        nc.sync.dma_start(cloc_sb[:],
                          inp["coords_loc"][:].rearrange("(s p) c -> p s c", p=128))
        cfac_sb = persist.tile([128, NSLAB, 1], F32)
        nc.sync.dma_start(cfac_sb[:],
                          inp["cfac"][:].rearrange("(s p) c -> p s c", p=128))
        wsc_sb = {}
        for i in range(1, 5):
            ktf = FDIM[i - 1] // 128
            t = persist.tile([128, ktf * 12], BF16, tag=f"wsc{i}", name=f"wsc{i}_sb")
            nc.sync.dma_start(t[:], inp[f"wsc{i}"][:])
            wsc_sb[i] = t
        biasrow_sb = {}
        for i in range(1, 4):
            CP = GAT_DIMS[i - 1][1] + 2
            t = persist.tile([1, CP], BF16, tag=f"biasrow{i}", name=f"biasrow{i}_sb")
            nc.sync.dma_start(t[:], inp[f"biasrow{i}"][:])
            biasrow_sb[i] = t

        # per-stage state
        SDall = persist.tile([128, NSLAB, 6], F32)
        SDrep = persist.tile([128, NSLAB, 48], F32)
        nc.vector.memset(SDrep[:], 0.0)
        CSTK = persist.tile([128, NSLAB, 8], F32)
        OUTC = persist.tile([128, NSLAB, 2], F32)

        # DRAM tables
        agin = {}
        feat = {}
        adsp = "Shared" if AG_MODE == "shared" else "Local"
        for s in range(1, 4):
            agin[s] = dram.tile([NLP, ROWB[s - 1]], BF16, tag=f"agin{s}",
                                name=f"agin{s}")
            feat[s] = dram.tile([NCORES * NLP, ROWB[s - 1]], BF16, tag=f"feat{s}",
                                name=f"feat{s}", addr_space=adsp)
        # stage-4 output table is transposed+compact: rows = 16 values
        # [c3(2), ss4(6), pad], cols = nodes; stage 5 reads it via ap_gather
        agin[4] = dram.tile([16, NLP], F32, tag="agin4", name="agin4")
        feat[4] = dram.tile([16 * NCORES, NLP], F32, tag="feat4",
                            name="feat4", addr_space=adsp)

        # barrier warm-up: a tiny collective issued first so the global
        # barrier (core launch skew) overlaps stage-1 compute
        warm_in = dram.tile([1, 128], BF16, name="warm_in")
        warm_out = dram.tile([NCORES, 128], BF16, name="warm_out")
        nc.sync.dma_start(warm_in[:], inp["onesrow"][:])
        nc.gpsimd.collective_compute(
            "AllGather", mybir.AluOpType.bypass, replica_groups=rg,
            ins=[warm_in[:].opt()], outs=[warm_out[:].opt()])

        nc.gpsimd.load_library(library_config.mlp)

        # =========================================================
        def selu_into(dst_ap, psum_ap, C):
            """dst = selu(psum[:, :C])  (bias already accumulated in psum)"""
            ex = e_pool.tile([128, C], F32, tag="selu_ex")
            nc.scalar.activation(ex[:], psum_ap, AF.Exp)
            m2 = e_pool.tile([128, C], F32, tag="selu_m2")
            nc.vector.tensor_scalar(m2[:], ex[:], LA, -LA, ALU.mult, ALU.add)
            m3 = e_pool.tile([128, C], F32, tag="selu_m3")
            nc.scalar.activation(m3[:], m2[:], AF.Relu, scale=-1.0)
            rp = e_pool.tile([128, C], F32, tag="selu_rp")
            nc.scalar.activation(rp[:], psum_ap, AF.Relu, scale=SELU_L)
            nc.vector.tensor_tensor(out=dst_ap, in0=rp[:], in1=m3[:],
                                    op=ALU.subtract)

        def ag_chunk(stage, c):
            lo, hi = CHROWS * c, CHROWS * (c + 1)
            fl, fh = NCORES * lo, NCORES * hi
            nc.gpsimd.collective_compute(
                "AllGather", mybir.AluOpType.bypass, replica_groups=rg,
                ins=[agin[stage][lo:hi, :].opt()],
                outs=[feat[stage][fl:fh, :].opt()])

        def ag_full(stage):
            nc.gpsimd.collective_compute(
                "AllGather", mybir.AluOpType.bypass, replica_groups=rg,
                ins=[agin[stage][:].opt()],
                outs=[feat[stage][:].opt()])
            if GATHER_MODE == "hoisted":
                # probe DMA is gated on AG completion by the framework;
                # its then_inc gives gpsimd a waitable completion signal
                probe = small.tile([1, 2], BF16, tag="probe")
                nc.sync.dma_start(probe[:], feat[stage][0:1, 0:2]) \
                    .then_inc(ag_sem, 16)

        def produce(stage, s, psum_f):
            """psum_f -> FN (bf16 feat row) for slab s; scores; ship."""
            din_out = FDIM[stage - 1]
            rowlen = ROWB[stage - 1]
            FN = fn_pool.tile([128, max(rowlen, din_out)], BF16, tag="FN")
            if stage == 1:
                nc.vector.tensor_copy(FN[:, 0:2], cloc_sb[:, s, :])
                nc.vector.tensor_copy(CSTK[:, s, 6:8], cloc_sb[:, s, :])
                selu_into(FN[:, 2:256], psum_f[:, 0:254], 254)
            else:
                C = GAT_DIMS[stage - 2][1]
                cnode = CSTK[:, s, 10 - 2 * stage:12 - 2 * stage]
                tcf = small.tile([128, 2], F32, tag="coord_t")
                nc.vector.tensor_scalar(tcf[:], cnode, cfac_sb[:, s, :], None,
                                        mybir.AluOpType.mult)
                cnw = small.tile([128, 2], F32, tag="cnw")
                nc.vector.tensor_add(cnw[:], psum_f[:, C:C + 2], tcf[:])
                nc.vector.tensor_copy(FN[:, 0:2], cnw[:])
                nc.vector.tensor_copy(CSTK[:, s, 8 - 2 * stage:10 - 2 * stage],
                                      cnw[:])
                nstk = 2 * (stage - 1)
                nc.vector.tensor_copy(FN[:, 2:2 + nstk],
                                      CSTK[:, s, 10 - 2 * stage:8])
                selu_into(FN[:, 2 + nstk:2 + nstk + C], psum_f[:, 0:C], C)
            # scores for GAT layer `stage` (FN holds full t_{stage})
            ktf = din_out // 128
            psum_s = ps_sm.tile([128, 12], F32, tag="pssmall", bufs=1)
            for kt in range(ktf):
                pt = ps_sm.tile([128, 128], BF16, tag="pt", bufs=1)
                nc.tensor.transpose(pt[:], FN[:, 128 * kt:128 * (kt + 1)], ident_sb[:])
                fnt = fnt_pool.tile([128, 128], BF16, tag="fnt")
                if kt % 2 == 0:
                    nc.vector.tensor_copy(fnt[:], pt[:])
                else:
                    nc.scalar.copy(fnt[:], pt[:])
                nc.tensor.matmul(psum_s[:], fnt[:],
                                 wsc_sb[stage][:, 12 * kt:12 * (kt + 1)],
                                 start=(kt == 0), stop=(kt == ktf - 1))
            nc.scalar.copy(SDall[:, s, :], psum_s[:, 6:12])
            if stage == 4:
                # compact transposed stage-5 table: [c3(2), ss4(6), pad]^T
                FC = fn_pool.tile([128, 16], F32, tag="FC")
                nc.vector.tensor_copy(FC[:, 0:2], cnw[:])
                nc.vector.tensor_copy(FC[:, 2:8], psum_s[:, 0:6])
                nc.vector.memset(FC[:, 8:16], 0.0)
                pt4 = ps_sm.tile([16, 128], F32, tag="pt", bufs=1)
                nc.tensor.transpose(pt4[:], FC[:], identf_sb[:])
                FCt = fn_pool.tile([16, 128], F32, tag="FCt")
                nc.vector.tensor_copy(FCt[:], pt4[:])
                nc.sync.dma_start(agin[stage][:, 128 * s:128 * (s + 1)], FCt[:])
            else:
                nc.vector.tensor_copy(FN[:, din_out:din_out + 6], psum_s[:, 0:6])
                nc.sync.dma_start(agin[stage][128 * s:128 * (s + 1), :],
                                  FN[:, 0:rowlen])
            if AG_MODE == "chunked":
                if s % 2 == 1:
                    ag_chunk(stage, s // 2)
            elif s == NSLAB - 1:
                ag_full(stage)

        def sdrep_build():
            for b in range(8):
                nc.sync.dma_start(SDrep[16 * b:16 * (b + 1), :, 6 * b:6 * (b + 1)],
                                  SDall[16 * b:16 * (b + 1), :, :])

        # ---- gather machinery (hoisted mode) ----
        dma_sems = [nc.alloc_semaphore(f"swdge_dma{q}") for q in range(4)]
        ag_sem = nc.alloc_semaphore("ag_done")
        if GATHER_MODE == "hoisted":
            nc.sync.sem_clear(ag_sem)
            for q in range(4):
                nc.sync.sem_clear(dma_sems[q])
        fg_tiles = {st: {} for st in range(2, 6)}

        def emit_prep(stage, s):
            """Desc-gen for `stage` slab s (s in 0..3) on SWDGE queue s."""
            rowlen = ROWB[stage - 2]
            Fg = fg_pool.tile([128, 8, rowlen], BF16, tag="Fgp", bufs=4,
                              name=f"Fgp{stage}_{s}")
            nc.gpsimd.dma_gather(Fg[:], feat[stage - 1][:],
                                 srcidx_sb[:, 64 * s:64 * (s + 1)],
                                 1024, 1024, rowlen,
                                 prepare_only=True, sem=dma_sems[s], queue_num=s)
            fg_tiles[stage][s] = Fg

        def emit_indirect(stage, s):
            rowlen = ROWB[stage - 2]
            Fg = fg_pool.tile([128, 8, rowlen], BF16, tag="Fg",
                              name=f"Fgi{stage}_{s}")
            for b in range(8):
                nc.gpsimd.indirect_dma_start(
                    out=Fg[:, b, :], out_offset=None,
                    in_=feat[stage - 1][:],
                    in_offset=bass.IndirectOffsetOnAxis(
                        ap=sidx32_sb[:, 8 * s + b:8 * s + b + 1], axis=0))
            fg_tiles[stage][s] = Fg

        # =========================================================
        # STAGE 1: feat1 from data
        for s in range(NSLAB):
            psum_f = ps_f.tile([128, 254], F32, tag="psum_f")
            nc.tensor.matmul(psum_f[:], dataT_sb[0:10, 128 * s:128 * (s + 1)],
                             linW_sb[0:10, :], start=True, stop=False)
            nc.tensor.matmul(psum_f[:], onesrow_sb[:], bias1row_sb[:],
                             start=False, stop=True)
            produce(1, s, psum_f)
        sdrep_build()

        # =========================================================
        # STAGES 2..5: GAT layers 1..4
        for stage in range(2, 6):
            g = stage - 1
            din, C = GAT_DIMS[g - 1]
            kt = din // 128
            CP = 2 if g == 4 else C + 2
            rowlen = ROWB[g - 1]
            sco = 2 if g == 4 else din   # score offset within table row
            ftab = feat[g]

            wp_t = wp_pool.tile([128, kt * 6 * CP], BF16, tag="wp")
            nc.sync.dma_start(wp_t[:], inp[f"wp{g}"][:])

            if g == 4:
                # stage 5: load ap_gather ucode + the transposed compact table
                nc.gpsimd.load_library(library_config.ap_gather)
                nc.sync.dma_start(
                    tabT_sb[:].rearrange("v (r i) -> v r i", r=NCORES),
                    feat[4][:].rearrange("(r v) i -> v r i", v=16))
            elif GATHER_MODE == "hoisted":
                # desc-gen for slabs 0-3 runs during the AllGather flight;
                # triggers fire the prepped gathers once the table lands
                for q in range(4):
                    emit_prep(stage, q)
                nc.gpsimd.wait_ge(ag_sem, 16 * (stage - 1))
                for q in range(4):
                    nc.gpsimd.trigger_dma(count=None, queue_num=q)
            elif GATHER_MODE == "indirect":
                emit_indirect(stage, 0)
                emit_indirect(stage, 1)

            for s in range(NSLAB):
                # ---- gather ----
                if g == 4:
                    # per-edge [c3|ss4] via SBUF ap_gather + PE transposes
                    apg = e_pool.tile([16, 1024], F32, tag="apg", bufs=4)
                    nc.gpsimd.ap_gather(apg[:], tabT_sb[:],
                                        srcidx_sb[0:16, 64 * s:64 * (s + 1)],
                                        16, NCORES * NLP, 1, 1024)
                    ptb = ps_sm.tile([128, 128], F32, tag="pbc")
                    for b in range(8):
                        nc.tensor.transpose(ptb[:, 16 * b:16 * (b + 1)],
                                            apg[:, 128 * b:128 * (b + 1)],
                                            identf_sb[0:16, 0:16])
                    TB = m_pool.tile([128, 8, 16], BF16, tag="TB", bufs=3)
                    nc.vector.tensor_copy(
                        TB[:].rearrange("p b v -> p (b v)"), ptb[:])
                    Fg = None
                elif GATHER_MODE == "dma_gather":
                    Fg = fg_pool.tile([128, 8, rowlen], BF16, tag="Fg")
                    nc.gpsimd.dma_gather(Fg[:], ftab[:],
                                         srcidx_sb[:, 64 * s:64 * (s + 1)],
                                         1024, 1024, rowlen)
                else:
                    Fg = fg_tiles[stage][s]

                # ---- edge phase (merged one-hot matmuls) ----
                pbc = ps_sm.tile([128, 48], F32, tag="pbc")
                nc.tensor.matmul(pbc[:], p0stk_sb[:, 128 * s:128 * (s + 1)],
                                 SDrep[:, s, :], start=True, stop=True)
                sc_ap = TB[:, :, 2:8] if g == 4 else Fg[:, :, sco:sco + 6]
                E_sl = e_pool.tile([128, 8, 6], F32, tag="E_sl")
                nc.vector.tensor_add(E_sl[:], sc_ap,
                                     pbc[:].rearrange("p (b h) -> p b h", b=8))
                t_lr = e_pool.tile([128, 8, 6], F32, tag="t_lr")
                nc.scalar.activation(t_lr[:], E_sl[:], AF.Copy, scale=0.2)
                E2 = e_pool.tile([128, 8, 6], F32, tag="E2")
                nc.vector.tensor_tensor(out=E2[:], in0=E_sl[:], in1=t_lr[:],
                                        op=ALU.max)
                EX = e_pool.tile([128, 8, 6], F32, tag="EX")
                nc.scalar.activation(EX[:], E2[:], AF.Exp)

                pdn = ps_sm.tile([128, 48], F32, tag="pbc")
                nc.tensor.matmul(pdn[:], p0_sb[:, 128 * s:128 * (s + 1)],
                                 EX[:].rearrange("p b h -> p (b h)"),
                                 start=True, stop=True)
                dple = e_pool.tile([128, 48], F32, tag="dple")
                nc.vector.tensor_scalar_add(dple[:], pdn[:], 1e-16)
                rd = e_pool.tile([128, 48], F32, tag="rd")
                nc.vector.reciprocal(rd[:], dple[:])
                rdm = e_pool.tile([128, 48], F32, tag="rdm")
                nc.vector.tensor_mul(rdm[:], rd[:], diagmask_sb[:])
                prd = ps_sm.tile([128, 48], F32, tag="pbc")
                nc.tensor.matmul(prd[:], p0stk_sb[:, 128 * s:128 * (s + 1)],
                                 rdm[:], start=True, stop=True)
                A_sl = e_pool.tile([128, 8, 6], BF16, tag="A_sl")
                nc.vector.tensor_mul(A_sl[:], EX[:],
                                     prd[:].rearrange("p (b h) -> p b h", b=8))

                M_sl = m_pool.tile([128, 8, 96], BF16, tag="M_sl")
                nc.vector.tensor_mul(
                    M_sl[:].rearrange("p b (h d) -> p b h d", h=6),
                    p0rep_sb[:, 96 * 8 * s:96 * 8 * (s + 1)]
                    .rearrange("p (b h d) -> p b h d", b=8, h=6),
                    A_sl[:].unsqueeze(3).broadcast_to([128, 8, 6, 16]))

                # ---- scatter: Gt cols [ds][h*128 + b*16 + dl] ----
                if g == 4:
                    Gt5 = gt_pool.tile([16, 6, 128], BF16, tag="Gt")
                    for half in range(2):
                        pg5 = ps_gt.tile([16, 4, 96], F32, tag="pgt")
                        for bb in range(4):
                            b = 4 * half + bb
                            nc.tensor.matmul(pg5[:, bb, :], TB[:, b, :],
                                             M_sl[:, b, :],
                                             start=True, stop=True)
                        eng_copy = (nc.vector.tensor_copy if half == 0
                                    else nc.scalar.copy)
                        eng_copy(
                            Gt5[:].rearrange("f h (b2 e) -> f h b2 e",
                                             b2=8)[:, :, 4 * half:4 * half + 4, :],
                            pg5[:].rearrange("f b2 (h e) -> f h b2 e", h=6))
                    psum_f = ps_f.tile([128, CP], F32, tag="psum_f")
                    for h in range(6):
                        nc.tensor.matmul(
                            psum_f[:], Gt5[:, h, :],
                            wp_t[0:16, 2 * h:2 * (h + 1)],
                            start=(h == 0), stop=(h == 5))
                else:
                    Gt = gt_pool.tile([128, kt, 768], BF16, tag="Gt")
                    for b in range(8):
                        pgt = ps_gt.tile([128, kt * 96], F32, tag="pgt")
                        for ds in range(kt):
                            nc.tensor.matmul(pgt[:, 96 * ds:96 * (ds + 1)],
                                             Fg[:, b, 128 * ds:128 * (ds + 1)],
                                             M_sl[:, b, :], start=True, stop=True)
                        eng_copy = (nc.vector.tensor_copy if b % 2 == 0
                                    else nc.scalar.copy)
                        eng_copy(
                            Gt[:].rearrange("p d (h2 b2 e) -> p d h2 b2 e",
                                            h2=6, b2=8)[:, :, :, b, :],
                            pgt[:].rearrange("p (d h2 e) -> p d h2 e", d=kt, h2=6))

                    # ---- feature matmul (bias row first) ----
                    psum_f = ps_f.tile([128, CP], F32, tag="psum_f")
                    nmm = kt * 6
                    i_mm = 0
                    nc.tensor.matmul(psum_f[:], onesrow_sb[:], biasrow_sb[g][:],
                                     start=True, stop=False)
                    for ds in range(kt):
                        for h in range(6):
                            nc.tensor.matmul(psum_f[:], Gt[:, ds, 128 * h:128 * (h + 1)],
                                             wp_t[:, (ds * 6 + h) * CP:(ds * 6 + h + 1) * CP],
                                             start=False,
                                             stop=(i_mm == nmm - 1))
                            i_mm += 1

                # ---- postprocess ----
                if stage < 5:
                    produce(stage, s, psum_f)
                else:
                    cnode = CSTK[:, s, 2:4]
                    tcf = small.tile([128, 2], F32, tag="coord_t")
                    nc.vector.tensor_scalar(tcf[:], cnode, cfac_sb[:, s, :], None,
                                            mybir.AluOpType.mult)
                    nc.vector.tensor_add(OUTC[:, s, :], psum_f[:, 0:2], tcf[:])

                if g != 4 and GATHER_MODE == "hoisted" and 2 <= s <= 7:
                    emit_indirect(stage, s + 2)
                elif g != 4 and GATHER_MODE == "indirect" and s <= 7:
                    emit_indirect(stage, s + 2)

            if stage < 5:
                sdrep_build()

        nc.sync.dma_start(out_t[:].rearrange("(s p) c -> p s c", p=128), OUTC[:])

    nc.compile()
    return nc


# ================================================================
def _host_prep(inputs):
    data = np.asarray(inputs["data"], np.float32)
    eidx = np.asarray(inputs["edge_idx"])
    src_a, dst_a = eidx[0].astype(np.int64), eidx[1].astype(np.int64)
    order = np.argsort(dst_a, kind="stable")
    src_s, dst_s = src_a[order], dst_a[order]
    indeg = np.bincount(dst_a, minlength=N)

    shared = {}
    linW = np.zeros((16, 254), np.float32)
    linW[0:10] = np.asarray(inputs["lin_W"], np.float32)
    shared["linW"] = linW.astype(BF)
    shared["bias1row"] = np.asarray(inputs["lin_b"], np.float32)[None, :].astype(BF)
    shared["onesrow"] = np.ones((1, 128), BF)
    shared["ident"] = np.eye(128, dtype=BF)
    shared["identf"] = np.eye(128, dtype=np.float32)
    dmask = np.zeros((128, 48), np.float32)
    for b in range(8):
        dmask[16 * b:16 * (b + 1), 6 * b:6 * (b + 1)] = 1.0
    shared["diagmask"] = dmask
    for i in range(1, 5):
        din, C = GAT_DIMS[i - 1]
        kt = din // 128
        CP = 2 if i == 4 else C + 2
        W = np.asarray(inputs[f"W{i}"], np.float32).reshape(din, H, C)
        wp = np.zeros((din, H, CP), np.float32)
        if i < 4:
            wp[:, :, :C] = W / H
            brow = np.zeros((1, CP), np.float32)
            brow[0, :C] = np.asarray(inputs[f"b{i}"], np.float32)
            shared[f"biasrow{i}"] = brow.astype(BF)
        wp[0, :, CP - 2] = 1.0 / H
        wp[1, :, CP - 1] = 1.0 / H
        wp_h = np.zeros((128, kt * H * CP), np.float32)
        for ds in range(kt):
            wp_h[:, ds * H * CP:(ds + 1) * H * CP] = \
                wp[ds * 128:(ds + 1) * 128].reshape(128, H * CP)
        shared[f"wp{i}"] = wp_h.astype(BF)
        a_s = np.asarray(inputs[f"as{i}"], np.float32)
        a_d = np.asarray(inputs[f"ad{i}"], np.float32)
        ws = np.einsum("dhc,hc->dh", W, a_s)
        wd = np.einsum("dhc,hc->dh", W, a_d)
        wsc = np.concatenate([ws, wd], 1)
        ktf = FDIM[i - 1] // 128
        wsc_h = np.zeros((128, ktf * 12), np.float32)
        for ds in range(ktf):
            wsc_h[:, ds * 12:(ds + 1) * 12] = wsc[ds * 128:(ds + 1) * 128]
        shared[f"wsc{i}"] = wsc_h.astype(BF)

    in_maps = []
    for r in range(NCORES):
        m = dict(shared)
        lo, hi = NL * r, NL * (r + 1)
        dT = np.zeros((16, NLP), np.float32)
        dT[0:10, 0:NL] = data[lo:hi].T
        m["dataT"] = dT.astype(BF)
        cl = np.zeros((NLP, 2), np.float32)
        cl[0:NL] = data[lo:hi, 0:2]
        m["coords_loc"] = cl
        cf = np.ones((NLP, 1), np.float32)
        cf[0:NL, 0] = (indeg[lo:hi] == 0).astype(np.float32)
        m["cfac"] = cf

        sel = (dst_s >= lo) & (dst_s < hi)
        es, ed = src_s[sel], dst_s[sel] - lo
        p0 = np.zeros((128, NBLK * 16), np.float32)
        p0rep = np.zeros((128, NBLK * 96), np.float32)
        sidx = np.zeros((128, NBLK * 8), np.int16)
        sidx32 = np.zeros((128, NSLAB * 8), np.int32)
        blk = ed // 16
        for c in range(NBLK):
            emask = blk == c
            k = int(emask.sum())
            assert k <= CAP, f"block overflow core {r} blk {c}: {k}"
            if k == 0:
                continue
            srcs = es[emask]
            lds = ed[emask].astype(np.int64)
            dls = lds % 16
            p0c = np.zeros((128, 16), np.float32)
            p0c[np.arange(k), dls] = 1.0
            p0[:, 16 * c:16 * (c + 1)] = p0c
            p0rep[:, 96 * c:96 * (c + 1)] = np.tile(p0c, (1, 6))
            rr = srcs // NL
            ii = srcs % NL
            # feat table row for node (rr, ii)
            if AG_MODE == "chunked":
                ch = ii // CHROWS
                agrow = ch * (NCORES * CHROWS) + rr * CHROWS + (ii - ch * CHROWS)
            else:
                agrow = rr * NLP + ii
            fulls = np.zeros(128, np.int64)
            fulls[:k] = agrow
            s_i, b_i = c // 8, c % 8
            sidx32[:, 8 * s_i + b_i] = fulls
            ws_ = sidx[:, 64 * s_i:64 * (s_i + 1)]
            for e_i in range(128):
                gk = 128 * b_i + e_i
                ws_[gk % 16, gk // 16] = fulls[e_i]
        for s_i in range(NSLAB):
            w = sidx[:, 64 * s_i:64 * (s_i + 1)]
            w[16:] = np.tile(w[:16], (7, 1))
        # per-slab transposed one-hot for pbc/prd stationary
        p0stk = np.zeros((128, NSLAB * 128), np.float32)
        for s_i in range(NSLAB):
            p0stk[:, 128 * s_i:128 * (s_i + 1)] = \
                p0[:, 128 * s_i:128 * (s_i + 1)].T
        m["p0"] = p0
        m["p0stk"] = p0stk
        m["p0rep"] = p0rep.astype(BF)
        m["srcidx"] = sidx
        m["sidx32"] = sidx32
        in_maps.append(m)
    return in_maps


_NC_CACHE = None


def kernel(**inputs):
    global _NC_CACHE
    in_maps = _host_prep(inputs)
    if _NC_CACHE is None:
        _NC_CACHE = _build_nc()
    res = run_bass_kernel_spmd(_NC_CACHE, in_maps, core_ids=list(range(NCORES)))
    out = np.zeros((N, 2), np.float32)
    for r in range(NCORES):
        out[NL * r:NL * (r + 1)] = res.results[r]["out"][:NL]
    return out


# revision 78
# speedup vs baseline: 1.0784x; 1.0784x over previous
"""DeformGAT (4-layer) Trainium2 kernel — 8 NeuronCores SPMD.

Sharding: nodes in 8 contiguous blocks of 1250 (padded to 1280); edges are
assigned to their dst node's core (edges pre-sorted by dst on host). Weights
replicated. Per layer each core gathers src rows of the replicated bf16
feature table (dma_gather), computes per-edge softmax with merged one-hot
matmuls (dst-score broadcast / segment-sum / reciprocal broadcast each as a
single free-48 matmul per slab), aggregates with bf16 scatter matmuls,
applies the per-head output transform (head-mean, bias row and coordinate
displacement folded into an augmented weight matrix), then AllGathers its
produced rows in 5 chunks per stage (overlapped with compute) so every core
again holds the full table.
"""
import numpy as np
import ml_dtypes
from contextlib import ExitStack

import concourse.bacc as bacc
import concourse.bass as bass
import concourse.tile as tile
import concourse.mybir as mybir
from concourse import library_config
from concourse.bass_utils import run_bass_kernel_spmd

F32 = mybir.dt.float32
BF16 = mybir.dt.bfloat16
I16 = mybir.dt.int16
AF = mybir.ActivationFunctionType
ALU = mybir.AluOpType
BF = ml_dtypes.bfloat16

NCORES = 8
N = 10000
E = 60000
H = 6
NL = 1250          # real nodes per core
NLP = 1280         # padded nodes per core
NBLK = 80          # dst blocks of 16 per core
NSLAB = 10         # slabs of 128 dst nodes (8 blocks)
CAP = 128          # edge capacity per block (= chunk)
NCHUNK = 5         # AllGather chunks per stage
CHROWS = NLP // NCHUNK   # rows per AG chunk per core (256)

# GAT layer dims (din, C). Stage s (2..5) runs GAT s-1.
GAT_DIMS = [(256, 508), (512, 250), (256, 120), (128, 20)]
FDIM = [256, 512, 256, 128]          # feat_s dim produced by stage s
ROWB = [384, 640, 384, 128]          # bf16 table row elems (stage-4 table is
                                     # compact: [c3(2), ss4(6), pad] — stage 5
                                     # only needs src coords + src scores)

SELU_L = 1.0507009873554805
SELU_A = 1.6732632423543772
LA = SELU_L * SELU_A

AG_MODE = "shared"   # "shared": 1 full-table Shared-output AG per stage
                     # "chunked": NCHUNK Local AGs per stage
GATHER_MODE = "indirect"    # "dma_gather" | "indirect" | "hoisted"
                           # hoisted: slabs 0-3 desc-prepped during the prior
                           # stage (SWDGE queues 0-3, fired at stage start);
                           # slabs 4-9 in-loop indirect DMA, 2-slab lookahead


def _build_nc():
    nc = bacc.Bacc("TRN2", target_bir_lowering=False, debug=False,
                   num_devices=NCORES, num_swdge_queues=4,
                   dynamic_dma_scratch_size=32768)
    # ---------------- inputs ----------------
    inp = {}
    inp["dataT"] = nc.dram_tensor("dataT", [16, NLP], BF16, kind="ExternalInput")
    inp["coords_loc"] = nc.dram_tensor("coords_loc", [NLP, 2], F32, kind="ExternalInput")
    inp["cfac"] = nc.dram_tensor("cfac", [NLP, 1], F32, kind="ExternalInput")
    inp["srcidx"] = nc.dram_tensor("srcidx", [128, NBLK * 8], I16, kind="ExternalInput")
    inp["sidx32"] = nc.dram_tensor("sidx32", [128, NSLAB * 8], mybir.dt.int32,
                                   kind="ExternalInput")
    inp["p0"] = nc.dram_tensor("p0", [128, NBLK * 16], F32, kind="ExternalInput")
    inp["p0stk"] = nc.dram_tensor("p0stk", [128, NSLAB * 128], F32, kind="ExternalInput")
    inp["p0rep"] = nc.dram_tensor("p0rep", [128, NBLK * 96], BF16, kind="ExternalInput")
    inp["diagmask"] = nc.dram_tensor("diagmask", [128, 48], F32, kind="ExternalInput")
    inp["ident"] = nc.dram_tensor("ident", [128, 128], BF16, kind="ExternalInput")
    inp["linW"] = nc.dram_tensor("linW", [16, 254], BF16, kind="ExternalInput")
    inp["bias1row"] = nc.dram_tensor("bias1row", [1, 254], BF16, kind="ExternalInput")
    inp["onesrow"] = nc.dram_tensor("onesrow", [1, 128], BF16, kind="ExternalInput")
    for i in range(1, 5):
        din, C = GAT_DIMS[i - 1]
        kt = din // 128
        CP = 2 if i == 4 else C + 2
        inp[f"wp{i}"] = nc.dram_tensor(f"wp{i}", [128, kt * 6 * CP], BF16, kind="ExternalInput")
        if i < 4:
            inp[f"biasrow{i}"] = nc.dram_tensor(f"biasrow{i}", [1, CP], BF16, kind="ExternalInput")
        ktf = FDIM[i - 1] // 128
        inp[f"wsc{i}"] = nc.dram_tensor(f"wsc{i}", [128, ktf * 12], BF16, kind="ExternalInput")
    out_t = nc.dram_tensor("out", [NLP, 2], F32, kind="ExternalOutput")

    rg = [list(range(NCORES))]

    with tile.TileContext(nc) as tc, ExitStack() as ctx:
        persist = ctx.enter_context(tc.tile_pool(name="persist", bufs=1))
        dram = ctx.enter_context(tc.tile_pool(name="dram", bufs=1, space="DRAM"))
        fg_pool = ctx.enter_context(tc.tile_pool(name="fg", bufs=4))
        gt_pool = ctx.enter_context(tc.tile_pool(name="gt", bufs=2))
        m_pool = ctx.enter_context(tc.tile_pool(name="m", bufs=2))
        e_pool = ctx.enter_context(tc.tile_pool(name="ep", bufs=3))
        fn_pool = ctx.enter_context(tc.tile_pool(name="fn", bufs=3))
        fnt_pool = ctx.enter_context(tc.tile_pool(name="fnt", bufs=2))
        wp_pool = ctx.enter_context(tc.tile_pool(name="wp", bufs=1))
        small = ctx.enter_context(tc.tile_pool(name="small", bufs=3))
        ps_gt = ctx.enter_context(tc.tile_pool(name="psgt", bufs=2, space="PSUM"))
        ps_f = ctx.enter_context(tc.tile_pool(name="psf", bufs=2, space="PSUM"))
        ps_sm = ctx.enter_context(tc.tile_pool(name="pssm", bufs=2, space="PSUM"))

        # ------------- resident loads -------------
        srcidx_sb = persist.tile([128, NBLK * 8], I16)
        nc.sync.dma_start(srcidx_sb[:], inp["srcidx"][:])
        sidx32_sb = persist.tile([128, NSLAB * 8], mybir.dt.int32)
        nc.sync.dma_start(sidx32_sb[:], inp["sidx32"][:])
        p0_sb = persist.tile([128, NBLK * 16], F32)
        nc.sync.dma_start(p0_sb[:], inp["p0"][:])
        p0stk_sb = persist.tile([128, NSLAB * 128], F32)
        nc.sync.dma_start(p0stk_sb[:], inp["p0stk"][:])
        p0rep_sb = persist.tile([128, NBLK * 96], BF16)
        nc.sync.dma_start(p0rep_sb[:], inp["p0rep"][:])
        diagmask_sb = persist.tile([128, 48], F32)
        nc.sync.dma_start(diagmask_sb[:], inp["diagmask"][:])
        ident_sb = persist.tile([128, 128], BF16)
        nc.sync.dma_start(ident_sb[:], inp["ident"][:])
        dataT_sb = persist.tile([16, NLP], BF16)
        nc.sync.dma_start(dataT_sb[:], inp["dataT"][:])
        linW_sb = persist.tile([16, 254], BF16)
        nc.sync.dma_start(linW_sb[:], inp["linW"][:])
        bias1row_sb = persist.tile([1, 254], BF16)
        nc.sync.dma_start(bias1row_sb[:], inp["bias1row"][:])
        onesrow_sb = persist.tile([1, 128], BF16)
        nc.sync.dma_start(onesrow_sb[:], inp["onesrow"][:])
        cloc_sb = persist.tile([128, NSLAB, 2], F32)
        nc.sync.dma_start(cloc_sb[:],
                          inp["coords_loc"][:].rearrange("(s p) c -> p s c", p=128))
        cfac_sb = persist.tile([128, NSLAB, 1], F32)
        nc.sync.dma_start(cfac_sb[:],
                          inp["cfac"][:].rearrange("(s p) c -> p s c", p=128))
        wsc_sb = {}
        for i in range(1, 5):
            ktf = FDIM[i - 1] // 128
            t = persist.tile([128, ktf * 12], BF16, tag=f"wsc{i}", name=f"wsc{i}_sb")
            nc.sync.dma_start(t[:], inp[f"wsc{i}"][:])
            wsc_sb[i] = t
        biasrow_sb = {}
        for i in range(1, 4):
            CP = GAT_DIMS[i - 1][1] + 2
            t = persist.tile([1, CP], BF16, tag=f"biasrow{i}", name=f"biasrow{i}_sb")
            nc.sync.dma_start(t[:], inp[f"biasrow{i}"][:])
            biasrow_sb[i] = t

        # per-stage state
        SDall = persist.tile([128, NSLAB, 6], F32)
        SDrep = persist.tile([128, NSLAB, 48], F32)
        nc.vector.memset(SDrep[:], 0.0)
        CSTK = persist.tile([128, NSLAB, 8], F32)
        OUTC = persist.tile([128, NSLAB, 2], F32)

        # DRAM tables
        agin = {}
        feat = {}
        adsp = "Shared" if AG_MODE == "shared" else "Local"
        for s in range(1, 5):
            agin[s] = dram.tile([NLP, ROWB[s - 1]], BF16, tag=f"agin{s}",
                                name=f"agin{s}")
            feat[s] = dram.tile([NCORES * NLP, ROWB[s - 1]], BF16, tag=f"feat{s}",
                                name=f"feat{s}", addr_space=adsp)

        # barrier warm-up: a tiny collective issued first so the global
        # barrier (core launch skew) overlaps stage-1 compute
        warm_in = dram.tile([1, 128], BF16, name="warm_in")
        warm_out = dram.tile([NCORES, 128], BF16, name="warm_out")
        nc.sync.dma_start(warm_in[:], inp["onesrow"][:])
        nc.gpsimd.collective_compute(
            "AllGather", mybir.AluOpType.bypass, replica_groups=rg,
            ins=[warm_in[:].opt()], outs=[warm_out[:].opt()])

        nc.gpsimd.load_library(library_config.mlp)

        # =========================================================
        def selu_into(dst_ap, psum_ap, C):
            """dst = selu(psum[:, :C])  (bias already accumulated in psum)"""
            ex = e_pool.tile([128, C], F32, tag="selu_ex")
            nc.scalar.activation(ex[:], psum_ap, AF.Exp)
            m2 = e_pool.tile([128, C], F32, tag="selu_m2")
            nc.vector.tensor_scalar(m2[:], ex[:], LA, -LA, ALU.mult, ALU.add)
            m3 = e_pool.tile([128, C], F32, tag="selu_m3")
            nc.scalar.activation(m3[:], m2[:], AF.Relu, scale=-1.0)
            rp = e_pool.tile([128, C], F32, tag="selu_rp")
            nc.scalar.activation(rp[:], psum_ap, AF.Relu, scale=SELU_L)
            nc.vector.tensor_tensor(out=dst_ap, in0=rp[:], in1=m3[:],
                                    op=ALU.subtract)

        def ag_chunk(stage, c):
            lo, hi = CHROWS * c, CHROWS * (c + 1)
            fl, fh = NCORES * lo, NCORES * hi
            nc.gpsimd.collective_compute(
                "AllGather", mybir.AluOpType.bypass, replica_groups=rg,
                ins=[agin[stage][lo:hi, :].opt()],
                outs=[feat[stage][fl:fh, :].opt()])

        def ag_full(stage):
            nc.gpsimd.collective_compute(
                "AllGather", mybir.AluOpType.bypass, replica_groups=rg,
                ins=[agin[stage][:].opt()],
                outs=[feat[stage][:].opt()])
            if GATHER_MODE == "hoisted":
                # probe DMA is gated on AG completion by the framework;
                # its then_inc gives gpsimd a waitable completion signal
                probe = small.tile([1, 2], BF16, tag="probe")
                nc.sync.dma_start(probe[:], feat[stage][0:1, 0:2]) \
                    .then_inc(ag_sem, 16)

        def produce(stage, s, psum_f):
            """psum_f -> FN (bf16 feat row) for slab s; scores; ship."""
            din_out = FDIM[stage - 1]
            rowlen = ROWB[stage - 1]
            FN = fn_pool.tile([128, max(rowlen, din_out)], BF16, tag="FN")
            if stage == 1:
                nc.vector.tensor_copy(FN[:, 0:2], cloc_sb[:, s, :])
                nc.vector.tensor_copy(CSTK[:, s, 6:8], cloc_sb[:, s, :])
                selu_into(FN[:, 2:256], psum_f[:, 0:254], 254)
            else:
                C = GAT_DIMS[stage - 2][1]
                cnode = CSTK[:, s, 10 - 2 * stage:12 - 2 * stage]
                tcf = small.tile([128, 2], F32, tag="coord_t")
                nc.vector.tensor_scalar(tcf[:], cnode, cfac_sb[:, s, :], None,
                                        mybir.AluOpType.mult)
                cnw = small.tile([128, 2], F32, tag="cnw")
                nc.vector.tensor_add(cnw[:], psum_f[:, C:C + 2], tcf[:])
                nc.vector.tensor_copy(FN[:, 0:2], cnw[:])
                nc.vector.tensor_copy(CSTK[:, s, 8 - 2 * stage:10 - 2 * stage],
                                      cnw[:])
                nstk = 2 * (stage - 1)
                nc.vector.tensor_copy(FN[:, 2:2 + nstk],
                                      CSTK[:, s, 10 - 2 * stage:8])
                selu_into(FN[:, 2 + nstk:2 + nstk + C], psum_f[:, 0:C], C)
            # scores for GAT layer `stage` (FN holds full t_{stage})
            ktf = din_out // 128
            psum_s = ps_sm.tile([128, 12], F32, tag="pssmall", bufs=1)
            for kt in range(ktf):
                pt = ps_sm.tile([128, 128], BF16, tag="pt", bufs=1)
                nc.tensor.transpose(pt[:], FN[:, 128 * kt:128 * (kt + 1)], ident_sb[:])
                fnt = fnt_pool.tile([128, 128], BF16, tag="fnt")
                if kt % 2 == 0:
                    nc.vector.tensor_copy(fnt[:], pt[:])
                else:
                    nc.scalar.copy(fnt[:], pt[:])
                nc.tensor.matmul(psum_s[:], fnt[:],
                                 wsc_sb[stage][:, 12 * kt:12 * (kt + 1)],
                                 start=(kt == 0), stop=(kt == ktf - 1))
            nc.scalar.copy(SDall[:, s, :], psum_s[:, 6:12])
            if stage == 4:
                # compact stage-5 table row: [c3(2), ss4(6), zero pad]
                FC = fn_pool.tile([128, 128], BF16, tag="FC")
                nc.vector.tensor_copy(FC[:, 0:2], FN[:, 0:2])
                nc.vector.tensor_copy(FC[:, 2:8], psum_s[:, 0:6])
                nc.vector.memset(FC[:, 8:128], 0.0)
                nc.sync.dma_start(agin[stage][128 * s:128 * (s + 1), :], FC[:])
            else:
                nc.vector.tensor_copy(FN[:, din_out:din_out + 6], psum_s[:, 0:6])
                nc.sync.dma_start(agin[stage][128 * s:128 * (s + 1), :],
                                  FN[:, 0:rowlen])
            if AG_MODE == "chunked":
                if s % 2 == 1:
                    ag_chunk(stage, s // 2)
            elif s == NSLAB - 1:
                ag_full(stage)

        def sdrep_build():
            for b in range(8):
                nc.sync.dma_start(SDrep[16 * b:16 * (b + 1), :, 6 * b:6 * (b + 1)],
                                  SDall[16 * b:16 * (b + 1), :, :])

        # ---- gather machinery (hoisted mode) ----
        dma_sems = [nc.alloc_semaphore(f"swdge_dma{q}") for q in range(4)]
        ag_sem = nc.alloc_semaphore("ag_done")
        if GATHER_MODE == "hoisted":
            nc.sync.sem_clear(ag_sem)
            for q in range(4):
                nc.sync.sem_clear(dma_sems[q])
        fg_tiles = {st: {} for st in range(2, 6)}

        def emit_prep(stage, s):
            """Desc-gen for `stage` slab s (s in 0..3) on SWDGE queue s."""
            rowlen = ROWB[stage - 2]
            Fg = fg_pool.tile([128, 8, rowlen], BF16, tag="Fgp", bufs=4,
                              name=f"Fgp{stage}_{s}")
            nc.gpsimd.dma_gather(Fg[:], feat[stage - 1][:],
                                 srcidx_sb[:, 64 * s:64 * (s + 1)],
                                 1024, 1024, rowlen,
                                 prepare_only=True, sem=dma_sems[s], queue_num=s)
            fg_tiles[stage][s] = Fg

        def emit_indirect(stage, s):
            rowlen = ROWB[stage - 2]
            Fg = fg_pool.tile([128, 8, rowlen], BF16, tag="Fg",
                              name=f"Fgi{stage}_{s}")
            for b in range(8):
                nc.gpsimd.indirect_dma_start(
                    out=Fg[:, b, :], out_offset=None,
                    in_=feat[stage - 1][:],
                    in_offset=bass.IndirectOffsetOnAxis(
                        ap=sidx32_sb[:, 8 * s + b:8 * s + b + 1], axis=0))
            fg_tiles[stage][s] = Fg

        # =========================================================
        # STAGE 1: feat1 from data
        for s in range(NSLAB):
            psum_f = ps_f.tile([128, 254], F32, tag="psum_f")
            nc.tensor.matmul(psum_f[:], dataT_sb[0:10, 128 * s:128 * (s + 1)],
                             linW_sb[0:10, :], start=True, stop=False)
            nc.tensor.matmul(psum_f[:], onesrow_sb[:], bias1row_sb[:],
                             start=False, stop=True)
            produce(1, s, psum_f)
        sdrep_build()

        # =========================================================
        # STAGES 2..5: GAT layers 1..4
        for stage in range(2, 6):
            g = stage - 1
            din, C = GAT_DIMS[g - 1]
            kt = din // 128
            CP = 2 if g == 4 else C + 2
            rowlen = ROWB[g - 1]
            sco = 2 if g == 4 else din   # score offset within table row
            ftab = feat[g]

            wp_t = wp_pool.tile([128, kt * 6 * CP], BF16, tag="wp")
            nc.sync.dma_start(wp_t[:], inp[f"wp{g}"][:])

            if GATHER_MODE == "hoisted":
                # desc-gen for slabs 0-3 runs during the AllGather flight;
                # triggers fire the prepped gathers once the table lands
                for q in range(4):
                    emit_prep(stage, q)
                nc.gpsimd.wait_ge(ag_sem, 16 * (stage - 1))
                for q in range(4):
                    nc.gpsimd.trigger_dma(count=None, queue_num=q)
            elif GATHER_MODE == "indirect":
                emit_indirect(stage, 0)
                emit_indirect(stage, 1)

            for s in range(NSLAB):
                # ---- gather ----
                if GATHER_MODE == "dma_gather":
                    Fg = fg_pool.tile([128, 8, rowlen], BF16, tag="Fg")
                    nc.gpsimd.dma_gather(Fg[:], ftab[:],
                                         srcidx_sb[:, 64 * s:64 * (s + 1)],
                                         1024, 1024, rowlen)
                else:
                    Fg = fg_tiles[stage][s]

                # ---- edge phase (merged one-hot matmuls) ----
                pbc = ps_sm.tile([128, 48], F32, tag="pbc")
                nc.tensor.matmul(pbc[:], p0stk_sb[:, 128 * s:128 * (s + 1)],
                                 SDrep[:, s, :], start=True, stop=True)
                E_sl = e_pool.tile([128, 8, 6], F32, tag="E_sl")
                nc.vector.tensor_add(E_sl[:], Fg[:, :, sco:sco + 6],
                                     pbc[:].rearrange("p (b h) -> p b h", b=8))
                t_lr = e_pool.tile([128, 8, 6], F32, tag="t_lr")
                nc.scalar.activation(t_lr[:], E_sl[:], AF.Copy, scale=0.2)
                E2 = e_pool.tile([128, 8, 6], F32, tag="E2")
                nc.vector.tensor_tensor(out=E2[:], in0=E_sl[:], in1=t_lr[:],
                                        op=ALU.max)
                EX = e_pool.tile([128, 8, 6], F32, tag="EX")
                nc.scalar.activation(EX[:], E2[:], AF.Exp)

                pdn = ps_sm.tile([128, 48], F32, tag="pbc")
                nc.tensor.matmul(pdn[:], p0_sb[:, 128 * s:128 * (s + 1)],
                                 EX[:].rearrange("p b h -> p (b h)"),
                                 start=True, stop=True)
                dple = e_pool.tile([128, 48], F32, tag="dple")
                nc.vector.tensor_scalar_add(dple[:], pdn[:], 1e-16)
                rd = e_pool.tile([128, 48], F32, tag="rd")
                nc.vector.reciprocal(rd[:], dple[:])
                rdm = e_pool.tile([128, 48], F32, tag="rdm")
                nc.vector.tensor_mul(rdm[:], rd[:], diagmask_sb[:])
                prd = ps_sm.tile([128, 48], F32, tag="pbc")
                nc.tensor.matmul(prd[:], p0stk_sb[:, 128 * s:128 * (s + 1)],
                                 rdm[:], start=True, stop=True)
                A_sl = e_pool.tile([128, 8, 6], BF16, tag="A_sl")
                nc.vector.tensor_mul(A_sl[:], EX[:],
                                     prd[:].rearrange("p (b h) -> p b h", b=8))

                M_sl = m_pool.tile([128, 8, 96], BF16, tag="M_sl")
                nc.vector.tensor_mul(
                    M_sl[:].rearrange("p b (h d) -> p b h d", h=6),
                    p0rep_sb[:, 96 * 8 * s:96 * 8 * (s + 1)]
                    .rearrange("p (b h d) -> p b h d", b=8, h=6),
                    A_sl[:].unsqueeze(3).broadcast_to([128, 8, 6, 16]))

                # ---- scatter: Gt cols [ds][h*128 + b*16 + dl] ----
                Gt = gt_pool.tile([128, kt, 768], BF16, tag="Gt")
                for b in range(8):
                    pgt = ps_gt.tile([128, kt * 96], F32, tag="pgt")
                    for ds in range(kt):
                        nc.tensor.matmul(pgt[:, 96 * ds:96 * (ds + 1)],
                                         Fg[:, b, 128 * ds:128 * (ds + 1)],
                                         M_sl[:, b, :], start=True, stop=True)
                    eng_copy = (nc.vector.tensor_copy if b % 2 == 0
                                else nc.scalar.copy)
                    eng_copy(
                        Gt[:].rearrange("p d (h2 b2 e) -> p d h2 b2 e",
                                        h2=6, b2=8)[:, :, :, b, :],
                        pgt[:].rearrange("p (d h2 e) -> p d h2 e", d=kt, h2=6))

                # ---- feature matmul (bias row first) ----
                psum_f = ps_f.tile([128, CP], F32, tag="psum_f")
                nmm = kt * 6
                i_mm = 0
                if g < 4:
                    nc.tensor.matmul(psum_f[:], onesrow_sb[:], biasrow_sb[g][:],
                                     start=True, stop=False)
                for ds in range(kt):
                    for h in range(6):
                        nc.tensor.matmul(psum_f[:], Gt[:, ds, 128 * h:128 * (h + 1)],
                                         wp_t[:, (ds * 6 + h) * CP:(ds * 6 + h + 1) * CP],
                                         start=(g == 4 and i_mm == 0),
                                         stop=(i_mm == nmm - 1))
                        i_mm += 1

                # ---- postprocess ----
                if stage < 5:
                    produce(stage, s, psum_f)
                else:
                    cnode = CSTK[:, s, 2:4]
                    tcf = small.tile([128, 2], F32, tag="coord_t")
                    nc.vector.tensor_scalar(tcf[:], cnode, cfac_sb[:, s, :], None,
                                            mybir.AluOpType.mult)
                    nc.vector.tensor_add(OUTC[:, s, :], psum_f[:, 0:2], tcf[:])

                if GATHER_MODE == "hoisted" and 2 <= s <= 7:
                    emit_indirect(stage, s + 2)
                elif GATHER_MODE == "indirect" and s <= 7:
                    emit_indirect(stage, s + 2)

            if stage < 5:
                sdrep_build()

        nc.sync.dma_start(out_t[:].rearrange("(s p) c -> p s c", p=128), OUTC[:])

    nc.compile()
    return nc


# ================================================================
def _host_prep(inputs):
    data = np.asarray(inputs["data"], np.float32)
    eidx = np.asarray(inputs["edge_idx"])
    src_a, dst_a = eidx[0].astype(np.int64), eidx[1].astype(np.int64)
    order = np.argsort(dst_a, kind="stable")
    src_s, dst_s = src_a[order], dst_a[order]
    indeg = np.bincount(dst_a, minlength=N)

    shared = {}
    linW = np.zeros((16, 254), np.float32)
    linW[0:10] = np.asarray(inputs["lin_W"], np.float32)
    shared["linW"] = linW.astype(BF)
    shared["bias1row"] = np.asarray(inputs["lin_b"], np.float32)[None, :].astype(BF)
    shared["onesrow"] = np.ones((1, 128), BF)
    shared["ident"] = np.eye(128, dtype=BF)
    dmask = np.zeros((128, 48), np.float32)
    for b in range(8):
        dmask[16 * b:16 * (b + 1), 6 * b:6 * (b + 1)] = 1.0
    shared["diagmask"] = dmask
    for i in range(1, 5):
        din, C = GAT_DIMS[i - 1]
        kt = din // 128
        CP = 2 if i == 4 else C + 2
        W = np.asarray(inputs[f"W{i}"], np.float32).reshape(din, H, C)
        wp = np.zeros((din, H, CP), np.float32)
        if i < 4:
            wp[:, :, :C] = W / H
            brow = np.zeros((1, CP), np.float32)
            brow[0, :C] = np.asarray(inputs[f"b{i}"], np.float32)
            shared[f"biasrow{i}"] = brow.astype(BF)
        wp[0, :, CP - 2] = 1.0 / H
        wp[1, :, CP - 1] = 1.0 / H
        wp_h = np.zeros((128, kt * H * CP), np.float32)
        for ds in range(kt):
            wp_h[:, ds * H * CP:(ds + 1) * H * CP] = \
                wp[ds * 128:(ds + 1) * 128].reshape(128, H * CP)
        shared[f"wp{i}"] = wp_h.astype(BF)
        a_s = np.asarray(inputs[f"as{i}"], np.float32)
        a_d = np.asarray(inputs[f"ad{i}"], np.float32)
        ws = np.einsum("dhc,hc->dh", W, a_s)
        wd = np.einsum("dhc,hc->dh", W, a_d)
        wsc = np.concatenate([ws, wd], 1)
        ktf = FDIM[i - 1] // 128
        wsc_h = np.zeros((128, ktf * 12), np.float32)
        for ds in range(ktf):
            wsc_h[:, ds * 12:(ds + 1) * 12] = wsc[ds * 128:(ds + 1) * 128]
        shared[f"wsc{i}"] = wsc_h.astype(BF)

    in_maps = []
    for r in range(NCORES):
        m = dict(shared)
        lo, hi = NL * r, NL * (r + 1)
        dT = np.zeros((16, NLP), np.float32)
        dT[0:10, 0:NL] = data[lo:hi].T
        m["dataT"] = dT.astype(BF)
        cl = np.zeros((NLP, 2), np.float32)
        cl[0:NL] = data[lo:hi, 0:2]
        m["coords_loc"] = cl
        cf = np.ones((NLP, 1), np.float32)
        cf[0:NL, 0] = (indeg[lo:hi] == 0).astype(np.float32)
        m["cfac"] = cf

        sel = (dst_s >= lo) & (dst_s < hi)
        es, ed = src_s[sel], dst_s[sel] - lo
        p0 = np.zeros((128, NBLK * 16), np.float32)
        p0rep = np.zeros((128, NBLK * 96), np.float32)
        sidx = np.zeros((128, NBLK * 8), np.int16)
        sidx32 = np.zeros((128, NSLAB * 8), np.int32)
        blk = ed // 16
        for c in range(NBLK):
            emask = blk == c
            k = int(emask.sum())
            assert k <= CAP, f"block overflow core {r} blk {c}: {k}"
            if k == 0:
                continue
            srcs = es[emask]
            lds = ed[emask].astype(np.int64)
            dls = lds % 16
            p0c = np.zeros((128, 16), np.float32)
            p0c[np.arange(k), dls] = 1.0
            p0[:, 16 * c:16 * (c + 1)] = p0c
            p0rep[:, 96 * c:96 * (c + 1)] = np.tile(p0c, (1, 6))
            rr = srcs // NL
            ii = srcs % NL
            # feat table row for node (rr, ii)
            if AG_MODE == "chunked":
                ch = ii // CHROWS
                agrow = ch * (NCORES * CHROWS) + rr * CHROWS + (ii - ch * CHROWS)
            else:
                agrow = rr * NLP + ii
            fulls = np.zeros(128, np.int64)
            fulls[:k] = agrow
            s_i, b_i = c // 8, c % 8
            sidx32[:, 8 * s_i + b_i] = fulls
            ws_ = sidx[:, 64 * s_i:64 * (s_i + 1)]
            for e_i in range(128):
                gk = 128 * b_i + e_i
                ws_[gk % 16, gk // 16] = fulls[e_i]
        for s_i in range(NSLAB):
            w = sidx[:, 64 * s_i:64 * (s_i + 1)]
            w[16:] = np.tile(w[:16], (7, 1))
        # per-slab transposed one-hot for pbc/prd stationary
        p0stk = np.zeros((128, NSLAB * 128), np.float32)
        for s_i in range(NSLAB):
            p0stk[:, 128 * s_i:128 * (s_i + 1)] = \
                p0[:, 128 * s_i:128 * (s_i + 1)].T
        m["p0"] = p0
        m["p0stk"] = p0stk
        m["p0rep"] = p0rep.astype(BF)
        m["srcidx"] = sidx
        m["sidx32"] = sidx32
        in_maps.append(m)
    return in_maps


_NC_CACHE = None


def kernel(**inputs):
    global _NC_CACHE
    in_maps = _host_prep(inputs)
    if _NC_CACHE is None:
        _NC_CACHE = _build_nc()
    res = run_bass_kernel_spmd(_NC_CACHE, in_maps, core_ids=list(range(NCORES)))
    out = np.zeros((N, 2), np.float32)
    for r in range(NCORES):
        out[NL * r:NL * (r + 1)] = res.results[r]["out"][:NL]
    return out


# revision 79
# speedup vs baseline: 1.0801x; 1.0016x over previous
"""DeformGAT (4-layer) Trainium2 kernel — 8 NeuronCores SPMD.

Sharding: nodes in 8 contiguous blocks of 1250 (padded to 1280); edges are
assigned to their dst node's core (edges pre-sorted by dst on host). Weights
replicated. Per layer each core gathers src rows of the replicated bf16
feature table (dma_gather), computes per-edge softmax with merged one-hot
matmuls (dst-score broadcast / segment-sum / reciprocal broadcast each as a
single free-48 matmul per slab), aggregates with bf16 scatter matmuls,
applies the per-head output transform (head-mean, bias row and coordinate
displacement folded into an augmented weight matrix), then AllGathers its
produced rows in 5 chunks per stage (overlapped with compute) so every core
again holds the full table.
"""
import numpy as np
import ml_dtypes
from contextlib import ExitStack

import concourse.bacc as bacc
import concourse.bass as bass
import concourse.tile as tile
import concourse.mybir as mybir
from concourse import library_config
from concourse.bass_utils import run_bass_kernel_spmd

F32 = mybir.dt.float32
BF16 = mybir.dt.bfloat16
I16 = mybir.dt.int16
AF = mybir.ActivationFunctionType
ALU = mybir.AluOpType
BF = ml_dtypes.bfloat16

NCORES = 8
N = 10000
E = 60000
H = 6
NL = 1250          # real nodes per core
NLP = 1280         # padded nodes per core
NBLK = 80          # dst blocks of 16 per core
NSLAB = 10         # slabs of 128 dst nodes (8 blocks)
CAP = 128          # edge capacity per block (= chunk)
NCHUNK = 5         # AllGather chunks per stage
CHROWS = NLP // NCHUNK   # rows per AG chunk per core (256)

# GAT layer dims (din, C). Stage s (2..5) runs GAT s-1.
GAT_DIMS = [(256, 508), (512, 250), (256, 120), (128, 20)]
FDIM = [256, 512, 256, 128]          # feat_s dim produced by stage s
ROWB = [384, 640, 384, 128]          # bf16 table row elems (stage-4 table is
                                     # compact: [c3(2), ss4(6), pad] — stage 5
                                     # only needs src coords + src scores)

SELU_L = 1.0507009873554805
SELU_A = 1.6732632423543772
LA = SELU_L * SELU_A

AG_MODE = "shared"   # "shared": 1 full-table Shared-output AG per stage
                     # "chunked": NCHUNK Local AGs per stage
GATHER_MODE = "indirect"    # "dma_gather" | "indirect" | "hoisted"
                           # hoisted: slabs 0-3 desc-prepped during the prior
                           # stage (SWDGE queues 0-3, fired at stage start);
                           # slabs 4-9 in-loop indirect DMA, 2-slab lookahead


def _build_nc():
    nc = bacc.Bacc("TRN2", target_bir_lowering=False, debug=False,
                   num_devices=NCORES, num_swdge_queues=4,
                   dynamic_dma_scratch_size=32768)
    # ---------------- inputs ----------------
    inp = {}
    inp["dataT"] = nc.dram_tensor("dataT", [16, NLP], BF16, kind="ExternalInput")
    inp["coords_loc"] = nc.dram_tensor("coords_loc", [NLP, 2], F32, kind="ExternalInput")
    inp["cfac"] = nc.dram_tensor("cfac", [NLP, 1], F32, kind="ExternalInput")
    inp["srcidx"] = nc.dram_tensor("srcidx", [128, NBLK * 8], I16, kind="ExternalInput")
    inp["sidx32"] = nc.dram_tensor("sidx32", [128, NSLAB * 8], mybir.dt.int32,
                                   kind="ExternalInput")
    inp["p0"] = nc.dram_tensor("p0", [128, NBLK * 16], F32, kind="ExternalInput")
    inp["p0stk"] = nc.dram_tensor("p0stk", [128, NSLAB * 128], F32, kind="ExternalInput")
    inp["p0rep"] = nc.dram_tensor("p0rep", [128, NBLK * 96], BF16, kind="ExternalInput")
    inp["diagmask"] = nc.dram_tensor("diagmask", [128, 48], F32, kind="ExternalInput")
    inp["ident"] = nc.dram_tensor("ident", [128, 128], BF16, kind="ExternalInput")
    inp["linW"] = nc.dram_tensor("linW", [16, 254], BF16, kind="ExternalInput")
    inp["bias1row"] = nc.dram_tensor("bias1row", [1, 254], BF16, kind="ExternalInput")
    inp["onesrow"] = nc.dram_tensor("onesrow", [1, 128], BF16, kind="ExternalInput")
    for i in range(1, 5):
        din, C = GAT_DIMS[i - 1]
        kt = din // 128
        CP = 2 if i == 4 else C + 2
        inp[f"wp{i}"] = nc.dram_tensor(f"wp{i}", [128, kt * 6 * CP], BF16, kind="ExternalInput")
        if i < 4:
            inp[f"biasrow{i}"] = nc.dram_tensor(f"biasrow{i}", [1, CP], BF16, kind="ExternalInput")
        ktf = FDIM[i - 1] // 128
        inp[f"wsc{i}"] = nc.dram_tensor(f"wsc{i}", [128, ktf * 12], BF16, kind="ExternalInput")
    out_t = nc.dram_tensor("out", [NLP, 2], F32, kind="ExternalOutput")

    rg = [list(range(NCORES))]

    with tile.TileContext(nc) as tc, ExitStack() as ctx:
        persist = ctx.enter_context(tc.tile_pool(name="persist", bufs=1))
        dram = ctx.enter_context(tc.tile_pool(name="dram", bufs=1, space="DRAM"))
        fg_pool = ctx.enter_context(tc.tile_pool(name="fg", bufs=3))
        gt_pool = ctx.enter_context(tc.tile_pool(name="gt", bufs=2))
        m_pool = ctx.enter_context(tc.tile_pool(name="m", bufs=2))
        e_pool = ctx.enter_context(tc.tile_pool(name="ep", bufs=2))
        fn_pool = ctx.enter_context(tc.tile_pool(name="fn", bufs=2))
        fnt_pool = ctx.enter_context(tc.tile_pool(name="fnt", bufs=2))
        wp_pool = ctx.enter_context(tc.tile_pool(name="wp", bufs=1))
        small = ctx.enter_context(tc.tile_pool(name="small", bufs=3))
        ps_gt = ctx.enter_context(tc.tile_pool(name="psgt", bufs=2, space="PSUM"))
        ps_f = ctx.enter_context(tc.tile_pool(name="psf", bufs=2, space="PSUM"))
        ps_sm = ctx.enter_context(tc.tile_pool(name="pssm", bufs=2, space="PSUM"))

        # ------------- resident loads -------------
        srcidx_sb = persist.tile([128, NBLK * 8], I16)
        nc.sync.dma_start(srcidx_sb[:], inp["srcidx"][:])
        sidx32_sb = persist.tile([128, NSLAB * 8], mybir.dt.int32)
        nc.sync.dma_start(sidx32_sb[:], inp["sidx32"][:])
        p0_sb = persist.tile([128, NBLK * 16], F32)
        nc.sync.dma_start(p0_sb[:], inp["p0"][:])
        p0stk_sb = persist.tile([128, NSLAB * 128], F32)
        nc.sync.dma_start(p0stk_sb[:], inp["p0stk"][:])
        p0rep_sb = persist.tile([128, NBLK * 96], BF16)
        nc.sync.dma_start(p0rep_sb[:], inp["p0rep"][:])
        diagmask_sb = persist.tile([128, 48], F32)
        nc.sync.dma_start(diagmask_sb[:], inp["diagmask"][:])
        ident_sb = persist.tile([128, 128], BF16)
        nc.sync.dma_start(ident_sb[:], inp["ident"][:])
        dataT_sb = persist.tile([16, NLP], BF16)
        nc.sync.dma_start(dataT_sb[:], inp["dataT"][:])
        linW_sb = persist.tile([16, 254], BF16)
        nc.sync.dma_start(linW_sb[:], inp["linW"][:])
        bias1row_sb = persist.tile([1, 254], BF16)
        nc.sync.dma_start(bias1row_sb[:], inp["bias1row"][:])
        onesrow_sb = persist.tile([1, 128], BF16)
        nc.sync.dma_start(onesrow_sb[:], inp["onesrow"][:])
        cloc_sb = persist.tile([128, NSLAB, 2], F32)
        nc.sync.dma_start(cloc_sb[:],
                          inp["coords_loc"][:].rearrange("(s p) c -> p s c", p=128))
        cfac_sb = persist.tile([128, NSLAB, 1], F32)
        nc.sync.dma_start(cfac_sb[:],
                          inp["cfac"][:].rearrange("(s p) c -> p s c", p=128))
        wsc_sb = {}
        for i in range(1, 5):
            ktf = FDIM[i - 1] // 128
            t = persist.tile([128, ktf * 12], BF16, tag=f"wsc{i}", name=f"wsc{i}_sb")
            nc.sync.dma_start(t[:], inp[f"wsc{i}"][:])
            wsc_sb[i] = t
        biasrow_sb = {}
        for i in range(1, 4):
            CP = GAT_DIMS[i - 1][1] + 2
            t = persist.tile([1, CP], BF16, tag=f"biasrow{i}", name=f"biasrow{i}_sb")
            nc.sync.dma_start(t[:], inp[f"biasrow{i}"][:])
            biasrow_sb[i] = t

        # per-stage state
        SDall = persist.tile([128, NSLAB, 6], F32)
        SDrep = persist.tile([128, NSLAB, 48], F32)
        nc.vector.memset(SDrep[:], 0.0)
        CSTK = persist.tile([128, NSLAB, 8], F32)
        OUTC = persist.tile([128, NSLAB, 2], F32)

        # DRAM tables
        agin = {}
        feat = {}
        adsp = "Shared" if AG_MODE == "shared" else "Local"
        for s in range(1, 5):
            agin[s] = dram.tile([NLP, ROWB[s - 1]], BF16, tag=f"agin{s}",
                                name=f"agin{s}")
            feat[s] = dram.tile([NCORES * NLP, ROWB[s - 1]], BF16, tag=f"feat{s}",
                                name=f"feat{s}", addr_space=adsp)

        # barrier warm-up: a tiny collective issued first so the global
        # barrier (core launch skew) overlaps stage-1 compute
        warm_in = dram.tile([1, 128], BF16, name="warm_in")
        warm_out = dram.tile([NCORES, 128], BF16, name="warm_out")
        nc.sync.dma_start(warm_in[:], inp["onesrow"][:])
        nc.gpsimd.collective_compute(
            "AllGather", mybir.AluOpType.bypass, replica_groups=rg,
            ins=[warm_in[:].opt()], outs=[warm_out[:].opt()])

        nc.gpsimd.load_library(library_config.mlp)

        # =========================================================
        def selu_into(dst_ap, psum_ap, C):
            """dst = selu(psum[:, :C])  (bias already accumulated in psum)"""
            ex = e_pool.tile([128, C], F32, tag="selu_ex")
            nc.scalar.activation(ex[:], psum_ap, AF.Exp)
            m2 = e_pool.tile([128, C], F32, tag="selu_m2")
            nc.vector.tensor_scalar(m2[:], ex[:], LA, -LA, ALU.mult, ALU.add)
            m3 = e_pool.tile([128, C], F32, tag="selu_m3")
            nc.scalar.activation(m3[:], m2[:], AF.Relu, scale=-1.0)
            rp = e_pool.tile([128, C], F32, tag="selu_rp")
            nc.scalar.activation(rp[:], psum_ap, AF.Relu, scale=SELU_L)
            nc.vector.tensor_tensor(out=dst_ap, in0=rp[:], in1=m3[:],
                                    op=ALU.subtract)

        def ag_chunk(stage, c):
            lo, hi = CHROWS * c, CHROWS * (c + 1)
            fl, fh = NCORES * lo, NCORES * hi
            nc.gpsimd.collective_compute(
                "AllGather", mybir.AluOpType.bypass, replica_groups=rg,
                ins=[agin[stage][lo:hi, :].opt()],
                outs=[feat[stage][fl:fh, :].opt()])

        def ag_full(stage):
            nc.gpsimd.collective_compute(
                "AllGather", mybir.AluOpType.bypass, replica_groups=rg,
                ins=[agin[stage][:].opt()],
                outs=[feat[stage][:].opt()])
            if GATHER_MODE == "hoisted":
                # probe DMA is gated on AG completion by the framework;
                # its then_inc gives gpsimd a waitable completion signal
                probe = small.tile([1, 2], BF16, tag="probe")
                nc.sync.dma_start(probe[:], feat[stage][0:1, 0:2]) \
                    .then_inc(ag_sem, 16)

        def produce(stage, s, psum_f):
            """psum_f -> FN (bf16 feat row) for slab s; scores; ship."""
            din_out = FDIM[stage - 1]
            rowlen = ROWB[stage - 1]
            FN = fn_pool.tile([128, max(rowlen, din_out)], BF16, tag="FN")
            if stage == 1:
                nc.vector.tensor_copy(FN[:, 0:2], cloc_sb[:, s, :])
                nc.vector.tensor_copy(CSTK[:, s, 6:8], cloc_sb[:, s, :])
                selu_into(FN[:, 2:256], psum_f[:, 0:254], 254)
            else:
                C = GAT_DIMS[stage - 2][1]
                cnode = CSTK[:, s, 10 - 2 * stage:12 - 2 * stage]
                tcf = small.tile([128, 2], F32, tag="coord_t")
                nc.vector.tensor_scalar(tcf[:], cnode, cfac_sb[:, s, :], None,
                                        mybir.AluOpType.mult)
                cnw = small.tile([128, 2], F32, tag="cnw")
                nc.vector.tensor_add(cnw[:], psum_f[:, C:C + 2], tcf[:])
                nc.vector.tensor_copy(FN[:, 0:2], cnw[:])
                nc.vector.tensor_copy(CSTK[:, s, 8 - 2 * stage:10 - 2 * stage],
                                      cnw[:])
                nstk = 2 * (stage - 1)
                nc.vector.tensor_copy(FN[:, 2:2 + nstk],
                                      CSTK[:, s, 10 - 2 * stage:8])
                selu_into(FN[:, 2 + nstk:2 + nstk + C], psum_f[:, 0:C], C)
            # scores for GAT layer `stage` (FN holds full t_{stage})
            ktf = din_out // 128
            psum_s = ps_sm.tile([128, 12], F32, tag="pssmall", bufs=1)
            for kt in range(ktf):
                pt = ps_sm.tile([128, 128], BF16, tag="pt", bufs=1)
                nc.tensor.transpose(pt[:], FN[:, 128 * kt:128 * (kt + 1)], ident_sb[:])
                fnt = fnt_pool.tile([128, 128], BF16, tag="fnt")
                if kt % 2 == 0:
                    nc.vector.tensor_copy(fnt[:], pt[:])
                else:
                    nc.scalar.copy(fnt[:], pt[:])
                nc.tensor.matmul(psum_s[:], fnt[:],
                                 wsc_sb[stage][:, 12 * kt:12 * (kt + 1)],
                                 start=(kt == 0), stop=(kt == ktf - 1))
            nc.scalar.copy(SDall[:, s, :], psum_s[:, 6:12])
            if stage == 4:
                # compact stage-5 table row: [c3(2), ss4(6), zero pad]
                FC = fn_pool.tile([128, 128], BF16, tag="FC")
                nc.vector.tensor_copy(FC[:, 0:2], FN[:, 0:2])
                nc.vector.tensor_copy(FC[:, 2:8], psum_s[:, 0:6])
                nc.vector.memset(FC[:, 8:128], 0.0)
                nc.sync.dma_start(agin[stage][128 * s:128 * (s + 1), :], FC[:])
            else:
                nc.vector.tensor_copy(FN[:, din_out:din_out + 6], psum_s[:, 0:6])
                nc.sync.dma_start(agin[stage][128 * s:128 * (s + 1), :],
                                  FN[:, 0:rowlen])
            if AG_MODE == "chunked":
                if s % 2 == 1:
                    ag_chunk(stage, s // 2)
            elif s == NSLAB - 1:
                ag_full(stage)

        def sdrep_build():
            for b in range(8):
                nc.sync.dma_start(SDrep[16 * b:16 * (b + 1), :, 6 * b:6 * (b + 1)],
                                  SDall[16 * b:16 * (b + 1), :, :])

        # ---- gather machinery (hoisted mode) ----
        dma_sems = [nc.alloc_semaphore(f"swdge_dma{q}") for q in range(4)]
        ag_sem = nc.alloc_semaphore("ag_done")
        if GATHER_MODE == "hoisted":
            nc.sync.sem_clear(ag_sem)
            for q in range(4):
                nc.sync.sem_clear(dma_sems[q])
        fg_tiles = {st: {} for st in range(2, 6)}

        def emit_prep(stage, s):
            """Desc-gen for `stage` slab s (s in 0..3) on SWDGE queue s."""
            rowlen = ROWB[stage - 2]
            Fg = fg_pool.tile([128, 8, rowlen], BF16, tag="Fgp", bufs=4,
                              name=f"Fgp{stage}_{s}")
            nc.gpsimd.dma_gather(Fg[:], feat[stage - 1][:],
                                 srcidx_sb[:, 64 * s:64 * (s + 1)],
                                 1024, 1024, rowlen,
                                 prepare_only=True, sem=dma_sems[s], queue_num=s)
            fg_tiles[stage][s] = Fg

        def emit_indirect(stage, s):
            rowlen = ROWB[stage - 2]
            Fg = fg_pool.tile([128, 8, rowlen], BF16, tag="Fg",
                              name=f"Fgi{stage}_{s}")
            for b in range(8):
                nc.gpsimd.indirect_dma_start(
                    out=Fg[:, b, :], out_offset=None,
                    in_=feat[stage - 1][:],
                    in_offset=bass.IndirectOffsetOnAxis(
                        ap=sidx32_sb[:, 8 * s + b:8 * s + b + 1], axis=0))
            fg_tiles[stage][s] = Fg

        # =========================================================
        # STAGE 1: feat1 from data
        for s in range(NSLAB):
            psum_f = ps_f.tile([128, 254], F32, tag="psum_f")
            nc.tensor.matmul(psum_f[:], dataT_sb[0:10, 128 * s:128 * (s + 1)],
                             linW_sb[0:10, :], start=True, stop=False)
            nc.tensor.matmul(psum_f[:], onesrow_sb[:], bias1row_sb[:],
                             start=False, stop=True)
            produce(1, s, psum_f)
        sdrep_build()

        # =========================================================
        # STAGES 2..5: GAT layers 1..4
        for stage in range(2, 6):
            g = stage - 1
            din, C = GAT_DIMS[g - 1]
            kt = din // 128
            CP = 2 if g == 4 else C + 2
            rowlen = ROWB[g - 1]
            sco = 2 if g == 4 else din   # score offset within table row
            ftab = feat[g]

            wp_t = wp_pool.tile([128, kt * 6 * CP], BF16, tag="wp")
            nc.sync.dma_start(wp_t[:], inp[f"wp{g}"][:])

            if GATHER_MODE == "hoisted":
                # desc-gen for slabs 0-3 runs during the AllGather flight;
                # triggers fire the prepped gathers once the table lands
                for q in range(4):
                    emit_prep(stage, q)
                nc.gpsimd.wait_ge(ag_sem, 16 * (stage - 1))
                for q in range(4):
                    nc.gpsimd.trigger_dma(count=None, queue_num=q)
            elif GATHER_MODE == "indirect":
                emit_indirect(stage, 0)
                emit_indirect(stage, 1)

            for s in range(NSLAB):
                # ---- gather ----
                if GATHER_MODE == "dma_gather":
                    Fg = fg_pool.tile([128, 8, rowlen], BF16, tag="Fg")
                    nc.gpsimd.dma_gather(Fg[:], ftab[:],
                                         srcidx_sb[:, 64 * s:64 * (s + 1)],
                                         1024, 1024, rowlen)
                else:
                    Fg = fg_tiles[stage][s]

                # ---- edge phase (merged one-hot matmuls) ----
                pbc = ps_sm.tile([128, 48], F32, tag="pbc")
                nc.tensor.matmul(pbc[:], p0stk_sb[:, 128 * s:128 * (s + 1)],
                                 SDrep[:, s, :], start=True, stop=True)
                E_sl = e_pool.tile([128, 8, 6], F32, tag="E_sl")
                nc.vector.tensor_add(E_sl[:], Fg[:, :, sco:sco + 6],
                                     pbc[:].rearrange("p (b h) -> p b h", b=8))
                t_lr = e_pool.tile([128, 8, 6], F32, tag="t_lr")
                nc.scalar.activation(t_lr[:], E_sl[:], AF.Copy, scale=0.2)
                E2 = e_pool.tile([128, 8, 6], F32, tag="E2")
                nc.vector.tensor_tensor(out=E2[:], in0=E_sl[:], in1=t_lr[:],
                                        op=ALU.max)
                EX = e_pool.tile([128, 8, 6], F32, tag="EX")
                nc.scalar.activation(EX[:], E2[:], AF.Exp)

                pdn = ps_sm.tile([128, 48], F32, tag="pbc")
                nc.tensor.matmul(pdn[:], p0_sb[:, 128 * s:128 * (s + 1)],
                                 EX[:].rearrange("p b h -> p (b h)"),
                                 start=True, stop=True)
                dple = e_pool.tile([128, 48], F32, tag="dple")
                nc.vector.tensor_scalar_add(dple[:], pdn[:], 1e-16)
                rd = e_pool.tile([128, 48], F32, tag="rd")
                nc.vector.reciprocal(rd[:], dple[:])
                rdm = e_pool.tile([128, 48], F32, tag="rdm")
                nc.vector.tensor_mul(rdm[:], rd[:], diagmask_sb[:])
                prd = ps_sm.tile([128, 48], F32, tag="pbc")
                nc.tensor.matmul(prd[:], p0stk_sb[:, 128 * s:128 * (s + 1)],
                                 rdm[:], start=True, stop=True)
                A_sl = e_pool.tile([128, 8, 6], BF16, tag="A_sl")
                nc.vector.tensor_mul(A_sl[:], EX[:],
                                     prd[:].rearrange("p (b h) -> p b h", b=8))

                M_sl = m_pool.tile([128, 8, 96], BF16, tag="M_sl")
                nc.vector.tensor_mul(
                    M_sl[:].rearrange("p b (h d) -> p b h d", h=6),
                    p0rep_sb[:, 96 * 8 * s:96 * 8 * (s + 1)]
                    .rearrange("p (b h d) -> p b h d", b=8, h=6),
                    A_sl[:].unsqueeze(3).broadcast_to([128, 8, 6, 16]))

                # ---- scatter: Gt cols [ds][h*128 + b*16 + dl] ----
                Gt = gt_pool.tile([128, kt, 768], BF16, tag="Gt")
                for b in range(8):
                    pgt = ps_gt.tile([128, kt * 96], F32, tag="pgt")
                    for ds in range(kt):
                        nc.tensor.matmul(pgt[:, 96 * ds:96 * (ds + 1)],
                                         Fg[:, b, 128 * ds:128 * (ds + 1)],
                                         M_sl[:, b, :], start=True, stop=True)
                    eng_copy = (nc.vector.tensor_copy if b % 2 == 0
                                else nc.scalar.copy)
                    eng_copy(
                        Gt[:].rearrange("p d (h2 b2 e) -> p d h2 b2 e",
                                        h2=6, b2=8)[:, :, :, b, :],
                        pgt[:].rearrange("p (d h2 e) -> p d h2 e", d=kt, h2=6))

                # ---- feature matmul (bias row first) ----
                psum_f = ps_f.tile([128, CP], F32, tag="psum_f")
                nmm = kt * 6
                i_mm = 0
                if g < 4:
                    nc.tensor.matmul(psum_f[:], onesrow_sb[:], biasrow_sb[g][:],
                                     start=True, stop=False)
                for ds in range(kt):
                    for h in range(6):
                        nc.tensor.matmul(psum_f[:], Gt[:, ds, 128 * h:128 * (h + 1)],
                                         wp_t[:, (ds * 6 + h) * CP:(ds * 6 + h + 1) * CP],
                                         start=(g == 4 and i_mm == 0),
                                         stop=(i_mm == nmm - 1))
                        i_mm += 1

                # ---- postprocess ----
                if stage < 5:
                    produce(stage, s, psum_f)
                else:
                    cnode = CSTK[:, s, 2:4]
                    tcf = small.tile([128, 2], F32, tag="coord_t")
                    nc.vector.tensor_scalar(tcf[:], cnode, cfac_sb[:, s, :], None,
                                            mybir.AluOpType.mult)
                    nc.vector.tensor_add(OUTC[:, s, :], psum_f[:, 0:2], tcf[:])

                if GATHER_MODE == "hoisted" and 2 <= s <= 7:
                    emit_indirect(stage, s + 2)
                elif GATHER_MODE == "indirect" and s <= 7:
                    emit_indirect(stage, s + 2)

            if stage < 5:
                sdrep_build()

        nc.sync.dma_start(out_t[:].rearrange("(s p) c -> p s c", p=128), OUTC[:])

    nc.compile()
    return nc


# ================================================================
def _host_prep(inputs):
    data = np.asarray(inputs["data"], np.float32)
    eidx = np.asarray(inputs["edge_idx"])
    src_a, dst_a = eidx[0].astype(np.int64), eidx[1].astype(np.int64)
    order = np.argsort(dst_a, kind="stable")
    src_s, dst_s = src_a[order], dst_a[order]
    indeg = np.bincount(dst_a, minlength=N)

    shared = {}
    linW = np.zeros((16, 254), np.float32)
    linW[0:10] = np.asarray(inputs["lin_W"], np.float32)
    shared["linW"] = linW.astype(BF)
    shared["bias1row"] = np.asarray(inputs["lin_b"], np.float32)[None, :].astype(BF)
    shared["onesrow"] = np.ones((1, 128), BF)
    shared["ident"] = np.eye(128, dtype=BF)
    dmask = np.zeros((128, 48), np.float32)
    for b in range(8):
        dmask[16 * b:16 * (b + 1), 6 * b:6 * (b + 1)] = 1.0
    shared["diagmask"] = dmask
    for i in range(1, 5):
        din, C = GAT_DIMS[i - 1]
        kt = din // 128
        CP = 2 if i == 4 else C + 2
        W = np.asarray(inputs[f"W{i}"], np.float32).reshape(din, H, C)
        wp = np.zeros((din, H, CP), np.float32)
        if i < 4:
            wp[:, :, :C] = W / H
            brow = np.zeros((1, CP), np.float32)
            brow[0, :C] = np.asarray(inputs[f"b{i}"], np.float32)
            shared[f"biasrow{i}"] = brow.astype(BF)
        wp[0, :, CP - 2] = 1.0 / H
        wp[1, :, CP - 1] = 1.0 / H
        wp_h = np.zeros((128, kt * H * CP), np.float32)
        for ds in range(kt):
            wp_h[:, ds * H * CP:(ds + 1) * H * CP] = \
                wp[ds * 128:(ds + 1) * 128].reshape(128, H * CP)
        shared[f"wp{i}"] = wp_h.astype(BF)
        a_s = np.asarray(inputs[f"as{i}"], np.float32)
        a_d = np.asarray(inputs[f"ad{i}"], np.float32)
        ws = np.einsum("dhc,hc->dh", W, a_s)
        wd = np.einsum("dhc,hc->dh", W, a_d)
        wsc = np.concatenate([ws, wd], 1)
        ktf = FDIM[i - 1] // 128
        wsc_h = np.zeros((128, ktf * 12), np.float32)
        for ds in range(ktf):
            wsc_h[:, ds * 12:(ds + 1) * 12] = wsc[ds * 128:(ds + 1) * 128]
        shared[f"wsc{i}"] = wsc_h.astype(BF)

    in_maps = []
    for r in range(NCORES):
        m = dict(shared)
        lo, hi = NL * r, NL * (r + 1)
        dT = np.zeros((16, NLP), np.float32)
        dT[0:10, 0:NL] = data[lo:hi].T
        m["dataT"] = dT.astype(BF)
        cl = np.zeros((NLP, 2), np.float32)
        cl[0:NL] = data[lo:hi, 0:2]
        m["coords_loc"] = cl
        cf = np.ones((NLP, 1), np.float32)
        cf[0:NL, 0] = (indeg[lo:hi] == 0).astype(np.float32)
        m["cfac"] = cf

        sel = (dst_s >= lo) & (dst_s < hi)
        es, ed = src_s[sel], dst_s[sel] - lo
        p0 = np.zeros((128, NBLK * 16), np.float32)
        p0rep = np.zeros((128, NBLK * 96), np.float32)
        sidx = np.zeros((128, NBLK * 8), np.int16)
        sidx32 = np.zeros((128, NSLAB * 8), np.int32)
        blk = ed // 16
        for c in range(NBLK):
            emask = blk == c
            k = int(emask.sum())
            assert k <= CAP, f"block overflow core {r} blk {c}: {k}"
            if k == 0:
                continue
            srcs = es[emask]
            lds = ed[emask].astype(np.int64)
            dls = lds % 16
            p0c = np.zeros((128, 16), np.float32)
            p0c[np.arange(k), dls] = 1.0
            p0[:, 16 * c:16 * (c + 1)] = p0c
            p0rep[:, 96 * c:96 * (c + 1)] = np.tile(p0c, (1, 6))
            rr = srcs // NL
            ii = srcs % NL
            # feat table row for node (rr, ii)
            if AG_MODE == "chunked":
                ch = ii // CHROWS
                agrow = ch * (NCORES * CHROWS) + rr * CHROWS + (ii - ch * CHROWS)
            else:
                agrow = rr * NLP + ii
            fulls = np.zeros(128, np.int64)
            fulls[:k] = agrow
            s_i, b_i = c // 8, c % 8
            sidx32[:, 8 * s_i + b_i] = fulls
            ws_ = sidx[:, 64 * s_i:64 * (s_i + 1)]
            for e_i in range(128):
                gk = 128 * b_i + e_i
                ws_[gk % 16, gk // 16] = fulls[e_i]
        for s_i in range(NSLAB):
            w = sidx[:, 64 * s_i:64 * (s_i + 1)]
            w[16:] = np.tile(w[:16], (7, 1))
        # per-slab transposed one-hot for pbc/prd stationary
        p0stk = np.zeros((128, NSLAB * 128), np.float32)
        for s_i in range(NSLAB):
            p0stk[:, 128 * s_i:128 * (s_i + 1)] = \
                p0[:, 128 * s_i:128 * (s_i + 1)].T
        m["p0"] = p0
        m["p0stk"] = p0stk
        m["p0rep"] = p0rep.astype(BF)
        m["srcidx"] = sidx
        m["sidx32"] = sidx32
        in_maps.append(m)
    return in_maps


_NC_CACHE = None


def kernel(**inputs):
    global _NC_CACHE
    in_maps = _host_prep(inputs)
    if _NC_CACHE is None:
        _NC_CACHE = _build_nc()
    res = run_bass_kernel_spmd(_NC_CACHE, in_maps, core_ids=list(range(NCORES)))
    out = np.zeros((N, 2), np.float32)
    for r in range(NCORES):
        out[NL * r:NL * (r + 1)] = res.results[r]["out"][:NL]
    return out


# revision 83
# speedup vs baseline: 1.2281x; 1.1371x over previous
"""DeformGAT (4-layer) Trainium2 kernel — 8 NeuronCores SPMD.

Sharding: nodes in 8 contiguous blocks of 1250 (padded to 1280); edges are
assigned to their dst node's core (edges pre-sorted by dst on host). Weights
replicated. Per layer each core gathers src rows of the replicated bf16
feature table (dma_gather), computes per-edge softmax with merged one-hot
matmuls (dst-score broadcast / segment-sum / reciprocal broadcast each as a
single free-48 matmul per slab), aggregates with bf16 scatter matmuls,
applies the per-head output transform (head-mean, bias row and coordinate
displacement folded into an augmented weight matrix), then AllGathers its
produced rows in 5 chunks per stage (overlapped with compute) so every core
again holds the full table.
"""
import numpy as np
import ml_dtypes
from contextlib import ExitStack

import concourse.bacc as bacc
import concourse.bass as bass
import concourse.tile as tile
import concourse.mybir as mybir
from concourse import library_config
from concourse.bass_utils import run_bass_kernel_spmd

F32 = mybir.dt.float32
BF16 = mybir.dt.bfloat16
I16 = mybir.dt.int16
AF = mybir.ActivationFunctionType
ALU = mybir.AluOpType
BF = ml_dtypes.bfloat16

NCORES = 8
N = 10000
E = 60000
H = 6
NL = 1250          # real nodes per core
NLP = 1280         # padded nodes per core
NBLK = 80          # dst blocks of 16 per core
NSLAB = 10         # slabs of 128 dst nodes (8 blocks)
CAP = 128          # edge capacity per block (= chunk)
NCHUNK = 5         # AllGather chunks per stage
CHROWS = NLP // NCHUNK   # rows per AG chunk per core (256)

# GAT layer dims (din, C). Stage s (2..5) runs GAT s-1.
GAT_DIMS = [(256, 508), (512, 250), (256, 120), (128, 20)]
FDIM = [256, 512, 256, 128]          # feat_s dim produced by stage s
ROWB = [384, 640, 384, 128]          # bf16 table row elems (stage-4 table is
                                     # compact: [c3(2), ss4(6), pad] — stage 5
                                     # only needs src coords + src scores)

SELU_L = 1.0507009873554805
SELU_A = 1.6732632423543772
LA = SELU_L * SELU_A

AG_MODE = "shared"   # "shared": 1 full-table Shared-output AG per stage
                     # "chunked": NCHUNK Local AGs per stage
GATHER_MODE = "indirect"    # "dma_gather" | "indirect" | "hoisted"
                           # hoisted: slabs 0-3 desc-prepped during the prior
                           # stage (SWDGE queues 0-3, fired at stage start);
                           # slabs 4-9 in-loop indirect DMA, 2-slab lookahead


def _build_nc():
    nc = bacc.Bacc("TRN2", target_bir_lowering=False, debug=False,
                   num_devices=NCORES, num_swdge_queues=4,
                   dynamic_dma_scratch_size=32768)
    # ---------------- inputs ----------------
    inp = {}
    inp["dataT"] = nc.dram_tensor("dataT", [16, NLP], BF16, kind="ExternalInput")
    inp["coords_loc"] = nc.dram_tensor("coords_loc", [NLP, 2], F32, kind="ExternalInput")
    inp["cfac"] = nc.dram_tensor("cfac", [NLP, 1], F32, kind="ExternalInput")
    inp["srcidx"] = nc.dram_tensor("srcidx", [128, NBLK * 8], I16, kind="ExternalInput")
    inp["sidx32"] = nc.dram_tensor("sidx32", [128, NSLAB * 8], mybir.dt.int32,
                                   kind="ExternalInput")
    inp["p0"] = nc.dram_tensor("p0", [128, NBLK * 16], F32, kind="ExternalInput")
    inp["p0stk"] = nc.dram_tensor("p0stk", [128, NSLAB * 128], F32, kind="ExternalInput")
    inp["p0rep"] = nc.dram_tensor("p0rep", [128, NBLK * 96], BF16, kind="ExternalInput")
    inp["diagmask"] = nc.dram_tensor("diagmask", [128, 48], F32, kind="ExternalInput")
    inp["ident"] = nc.dram_tensor("ident", [128, 128], BF16, kind="ExternalInput")
    inp["linW"] = nc.dram_tensor("linW", [16, 254], BF16, kind="ExternalInput")
    inp["bias1row"] = nc.dram_tensor("bias1row", [1, 254], BF16, kind="ExternalInput")
    inp["onesrow"] = nc.dram_tensor("onesrow", [1, 128], BF16, kind="ExternalInput")
    for i in range(1, 5):
        din, C = GAT_DIMS[i - 1]
        kt = din // 128
        CP = 2 if i == 4 else C + 2
        inp[f"wp{i}"] = nc.dram_tensor(f"wp{i}", [128, kt * 6 * CP], BF16, kind="ExternalInput")
        if i < 4:
            inp[f"biasrow{i}"] = nc.dram_tensor(f"biasrow{i}", [1, CP], BF16, kind="ExternalInput")
        ktf = FDIM[i - 1] // 128
        inp[f"wsc{i}"] = nc.dram_tensor(f"wsc{i}", [128, ktf * 12], BF16, kind="ExternalInput")
    out_t = nc.dram_tensor("out", [NLP, 2], F32, kind="ExternalOutput")

    rg = [list(range(NCORES))]

    with tile.TileContext(nc) as tc, ExitStack() as ctx:
        persist = ctx.enter_context(tc.tile_pool(name="persist", bufs=1))
        dram = ctx.enter_context(tc.tile_pool(name="dram", bufs=1, space="DRAM"))
        fg_pool = ctx.enter_context(tc.tile_pool(name="fg", bufs=3))
        gt_pool = ctx.enter_context(tc.tile_pool(name="gt", bufs=2))
        m_pool = ctx.enter_context(tc.tile_pool(name="m", bufs=2))
        e_pool = ctx.enter_context(tc.tile_pool(name="ep", bufs=2))
        fn_pool = ctx.enter_context(tc.tile_pool(name="fn", bufs=2))
        fnt_pool = ctx.enter_context(tc.tile_pool(name="fnt", bufs=2))
        wp_pool = ctx.enter_context(tc.tile_pool(name="wp", bufs=1))
        small = ctx.enter_context(tc.tile_pool(name="small", bufs=3))
        ps_gt = ctx.enter_context(tc.tile_pool(name="psgt", bufs=2, space="PSUM"))
        ps_f = ctx.enter_context(tc.tile_pool(name="psf", bufs=2, space="PSUM"))
        ps_sm = ctx.enter_context(tc.tile_pool(name="pssm", bufs=2, space="PSUM"))

        # ------------- resident loads -------------
        srcidx_sb = persist.tile([128, NBLK * 8], I16)
        nc.sync.dma_start(srcidx_sb[:], inp["srcidx"][:])
        sidx32_sb = persist.tile([128, NSLAB * 8], mybir.dt.int32)
        nc.sync.dma_start(sidx32_sb[:], inp["sidx32"][:])
        p0_sb = persist.tile([128, NBLK * 16], F32)
        nc.sync.dma_start(p0_sb[:], inp["p0"][:])
        p0stk_sb = persist.tile([128, NSLAB * 128], F32)
        nc.sync.dma_start(p0stk_sb[:], inp["p0stk"][:])
        p0rep_sb = persist.tile([128, NBLK * 96], BF16)
        nc.sync.dma_start(p0rep_sb[:], inp["p0rep"][:])
        diagmask_sb = persist.tile([128, 48], F32)
        nc.sync.dma_start(diagmask_sb[:], inp["diagmask"][:])
        ident_sb = persist.tile([128, 128], BF16)
        nc.sync.dma_start(ident_sb[:], inp["ident"][:])
        dataT_sb = persist.tile([16, NLP], BF16)
        nc.sync.dma_start(dataT_sb[:], inp["dataT"][:])
        linW_sb = persist.tile([16, 254], BF16)
        nc.sync.dma_start(linW_sb[:], inp["linW"][:])
        bias1row_sb = persist.tile([1, 254], BF16)
        nc.sync.dma_start(bias1row_sb[:], inp["bias1row"][:])
        onesrow_sb = persist.tile([1, 128], BF16)
        nc.sync.dma_start(onesrow_sb[:], inp["onesrow"][:])
        cloc_sb = persist.tile([128, NSLAB, 2], F32)
        nc.sync.dma_start(cloc_sb[:],
                          inp["coords_loc"][:].rearrange("(s p) c -> p s c", p=128))
        cfac_sb = persist.tile([128, NSLAB, 1], F32)
        nc.sync.dma_start(cfac_sb[:],
                          inp["cfac"][:].rearrange("(s p) c -> p s c", p=128))
        wsc_sb = {}
        for i in range(1, 5):
            ktf = FDIM[i - 1] // 128
            t = persist.tile([128, ktf * 12], BF16, tag=f"wsc{i}", name=f"wsc{i}_sb")
            nc.sync.dma_start(t[:], inp[f"wsc{i}"][:])
            wsc_sb[i] = t
        biasrow_sb = {}
        for i in range(1, 4):
            CP = GAT_DIMS[i - 1][1] + 2
            t = persist.tile([1, CP], BF16, tag=f"biasrow{i}", name=f"biasrow{i}_sb")
            nc.sync.dma_start(t[:], inp[f"biasrow{i}"][:])
            biasrow_sb[i] = t

        # per-stage state
        SDall = persist.tile([128, NSLAB, 6], F32)
        SDrep = persist.tile([128, NSLAB, 48], F32)
        nc.vector.memset(SDrep[:], 0.0)
        CSTK = persist.tile([128, NSLAB, 8], F32)
        OUTC = persist.tile([128, NSLAB, 2], F32)

        # DRAM tables
        agin = {}
        feat = {}
        adsp = "Shared" if AG_MODE == "shared" else "Local"
        for s in range(1, 5):
            agin[s] = dram.tile([NLP, ROWB[s - 1]], BF16, tag=f"agin{s}",
                                name=f"agin{s}")
            feat[s] = dram.tile([NCORES * NLP, ROWB[s - 1]], BF16, tag=f"feat{s}",
                                name=f"feat{s}", addr_space=adsp)

        # barrier warm-up: a tiny collective issued first so the global
        # barrier (core launch skew) overlaps stage-1 compute
        warm_in = dram.tile([1, 128], BF16, name="warm_in")
        warm_out = dram.tile([NCORES, 128], BF16, name="warm_out")
        nc.sync.dma_start(warm_in[:], inp["onesrow"][:])
        nc.gpsimd.collective_compute(
            "AllGather", mybir.AluOpType.bypass, replica_groups=rg,
            ins=[warm_in[:].opt()], outs=[warm_out[:].opt()])

        nc.gpsimd.load_library(library_config.mlp)

        # =========================================================
        def selu_into(dst_ap, psum_ap, C):
            """dst = selu(psum[:, :C])  (bias already accumulated in psum)"""
            ex = e_pool.tile([128, C], F32, tag="selu_ex")
            nc.scalar.activation(ex[:], psum_ap, AF.Exp)
            m2 = e_pool.tile([128, C], F32, tag="selu_m2")
            nc.vector.tensor_scalar(m2[:], ex[:], LA, -LA, ALU.mult, ALU.add)
            m3 = e_pool.tile([128, C], F32, tag="selu_m3")
            nc.scalar.activation(m3[:], m2[:], AF.Relu, scale=-1.0)
            rp = e_pool.tile([128, C], F32, tag="selu_rp")
            nc.scalar.activation(rp[:], psum_ap, AF.Relu, scale=SELU_L)
            nc.vector.tensor_tensor(out=dst_ap, in0=rp[:], in1=m3[:],
                                    op=ALU.subtract)

        def ag_chunk(stage, c):
            lo, hi = CHROWS * c, CHROWS * (c + 1)
            fl, fh = NCORES * lo, NCORES * hi
            nc.gpsimd.collective_compute(
                "AllGather", mybir.AluOpType.bypass, replica_groups=rg,
                ins=[agin[stage][lo:hi, :].opt()],
                outs=[feat[stage][fl:fh, :].opt()])

        def ag_full(stage):
            nc.gpsimd.collective_compute(
                "AllGather", mybir.AluOpType.bypass, replica_groups=rg,
                ins=[agin[stage][:].opt()],
                outs=[feat[stage][:].opt()])
            if GATHER_MODE == "hoisted":
                # probe DMA is gated on AG completion by the framework;
                # its then_inc gives gpsimd a waitable completion signal
                probe = small.tile([1, 2], BF16, tag="probe")
                nc.sync.dma_start(probe[:], feat[stage][0:1, 0:2]) \
                    .then_inc(ag_sem, 16)

        def produce(stage, s, psum_f):
            """psum_f -> FN (bf16 feat row) for slab s; scores; ship."""
            din_out = FDIM[stage - 1]
            rowlen = ROWB[stage - 1]
            FN = fn_pool.tile([128, max(rowlen, din_out)], BF16, tag="FN")
            if stage == 1:
                nc.vector.tensor_copy(FN[:, 0:2], cloc_sb[:, s, :])
                nc.vector.tensor_copy(CSTK[:, s, 6:8], cloc_sb[:, s, :])
                selu_into(FN[:, 2:256], psum_f[:, 0:254], 254)
            else:
                C = GAT_DIMS[stage - 2][1]
                cnode = CSTK[:, s, 10 - 2 * stage:12 - 2 * stage]
                tcf = small.tile([128, 2], F32, tag="coord_t")
                nc.vector.tensor_scalar(tcf[:], cnode, cfac_sb[:, s, :], None,
                                        mybir.AluOpType.mult)
                cnw = small.tile([128, 2], F32, tag="cnw")
                nc.vector.tensor_add(cnw[:], psum_f[:, C:C + 2], tcf[:])
                nc.vector.tensor_copy(FN[:, 0:2], cnw[:])
                nc.vector.tensor_copy(CSTK[:, s, 8 - 2 * stage:10 - 2 * stage],
                                      cnw[:])
                nstk = 2 * (stage - 1)
                nc.vector.tensor_copy(FN[:, 2:2 + nstk],
                                      CSTK[:, s, 10 - 2 * stage:8])
                selu_into(FN[:, 2 + nstk:2 + nstk + C], psum_f[:, 0:C], C)
            # scores for GAT layer `stage` (FN holds full t_{stage})
            ktf = din_out // 128
            psum_s = ps_sm.tile([128, 12], F32, tag="pssmall", bufs=1)
            for kt in range(ktf):
                pt = ps_sm.tile([128, 128], BF16, tag="pt", bufs=1)
                nc.tensor.transpose(pt[:], FN[:, 128 * kt:128 * (kt + 1)], ident_sb[:])
                fnt = fnt_pool.tile([128, 128], BF16, tag="fnt")
                if kt % 2 == 0:
                    nc.vector.tensor_copy(fnt[:], pt[:])
                else:
                    nc.scalar.copy(fnt[:], pt[:])
                nc.tensor.matmul(psum_s[:], fnt[:],
                                 wsc_sb[stage][:, 12 * kt:12 * (kt + 1)],
                                 start=(kt == 0), stop=(kt == ktf - 1))
            nc.scalar.copy(SDall[:, s, :], psum_s[:, 6:12])
            if stage == 4:
                # compact stage-5 table row: [c3(2), ss4(6), zero pad]
                FC = fn_pool.tile([128, 128], BF16, tag="FC")
                nc.vector.tensor_copy(FC[:, 0:2], FN[:, 0:2])
                nc.vector.tensor_copy(FC[:, 2:8], psum_s[:, 0:6])
                nc.vector.memset(FC[:, 8:128], 0.0)
                nc.sync.dma_start(agin[stage][128 * s:128 * (s + 1), :], FC[:])
            else:
                nc.vector.tensor_copy(FN[:, din_out:din_out + 6], psum_s[:, 0:6])
                nc.sync.dma_start(agin[stage][128 * s:128 * (s + 1), :],
                                  FN[:, 0:rowlen])
            if AG_MODE == "chunked":
                if s % 2 == 1:
                    ag_chunk(stage, s // 2)
            elif s == NSLAB - 1:
                ag_full(stage)

        def sdrep_build():
            for b in range(8):
                nc.sync.dma_start(SDrep[16 * b:16 * (b + 1), :, 6 * b:6 * (b + 1)],
                                  SDall[16 * b:16 * (b + 1), :, :])

        # ---- gather machinery (hoisted mode) ----
        dma_sems = [nc.alloc_semaphore(f"swdge_dma{q}") for q in range(4)]
        ag_sem = nc.alloc_semaphore("ag_done")
        if GATHER_MODE == "hoisted":
            nc.sync.sem_clear(ag_sem)
            for q in range(4):
                nc.sync.sem_clear(dma_sems[q])
        fg_tiles = {st: {} for st in range(2, 6)}

        def emit_prep(stage, s):
            """Desc-gen for `stage` slab s (s in 0..3) on SWDGE queue s."""
            rowlen = ROWB[stage - 2]
            Fg = fg_pool.tile([128, 8, rowlen], BF16, tag="Fgp", bufs=4,
                              name=f"Fgp{stage}_{s}")
            nc.gpsimd.dma_gather(Fg[:], feat[stage - 1][:],
                                 srcidx_sb[:, 64 * s:64 * (s + 1)],
                                 1024, 1024, rowlen,
                                 prepare_only=True, sem=dma_sems[s], queue_num=s)
            fg_tiles[stage][s] = Fg

        def emit_indirect(stage, s):
            rowlen = ROWB[stage - 2]
            Fg = fg_pool.tile([128, 8, rowlen], BF16, tag="Fg",
                              name=f"Fgi{stage}_{s}")
            for b in range(8):
                nc.gpsimd.indirect_dma_start(
                    out=Fg[:, b, :], out_offset=None,
                    in_=feat[stage - 1][:],
                    in_offset=bass.IndirectOffsetOnAxis(
                        ap=sidx32_sb[:, 8 * s + b:8 * s + b + 1], axis=0))
            fg_tiles[stage][s] = Fg

        # =========================================================
        # STAGE 1: feat1 from data
        for s in range(NSLAB):
            psum_f = ps_f.tile([128, 254], F32, tag="psum_f")
            nc.tensor.matmul(psum_f[:], dataT_sb[0:10, 128 * s:128 * (s + 1)],
                             linW_sb[0:10, :], start=True, stop=False)
            nc.tensor.matmul(psum_f[:], onesrow_sb[:], bias1row_sb[:],
                             start=False, stop=True)
            produce(1, s, psum_f)
        sdrep_build()

        # =========================================================
        # STAGES 2..5: GAT layers 1..4
        for stage in range(2, 6):
            g = stage - 1
            din, C = GAT_DIMS[g - 1]
            kt = din // 128
            CP = 2 if g == 4 else C + 2
            rowlen = ROWB[g - 1]
            sco = 2 if g == 4 else din   # score offset within table row
            ftab = feat[g]

            wp_t = wp_pool.tile([128, kt * 6 * CP], BF16, tag="wp")
            nc.sync.dma_start(wp_t[:], inp[f"wp{g}"][:])

            if GATHER_MODE == "hoisted":
                # desc-gen for slabs 0-3 runs during the AllGather flight;
                # triggers fire the prepped gathers once the table lands
                for q in range(4):
                    emit_prep(stage, q)
                nc.gpsimd.wait_ge(ag_sem, 16 * (stage - 1))
                for q in range(4):
                    nc.gpsimd.trigger_dma(count=None, queue_num=q)
            elif GATHER_MODE == "indirect":
                emit_indirect(stage, 0)
                emit_indirect(stage, 1)

            for s in range(NSLAB):
                # ---- gather ----
                if GATHER_MODE == "dma_gather":
                    Fg = fg_pool.tile([128, 8, rowlen], BF16, tag="Fg")
                    nc.gpsimd.dma_gather(Fg[:], ftab[:],
                                         srcidx_sb[:, 64 * s:64 * (s + 1)],
                                         1024, 1024, rowlen)
                else:
                    Fg = fg_tiles[stage][s]

                # ---- edge phase (merged one-hot matmuls) ----
                pbc = ps_sm.tile([128, 48], F32, tag="pbc")
                nc.tensor.matmul(pbc[:], p0stk_sb[:, 128 * s:128 * (s + 1)],
                                 SDrep[:, s, :], start=True, stop=True)
                E_sl = e_pool.tile([128, 8, 6], F32, tag="E_sl")
                nc.vector.tensor_add(E_sl[:], Fg[:, :, sco:sco + 6],
                                     pbc[:].rearrange("p (b h) -> p b h", b=8))
                t_lr = e_pool.tile([128, 8, 6], F32, tag="t_lr")
                nc.scalar.activation(t_lr[:], E_sl[:], AF.Copy, scale=0.2)
                E2 = e_pool.tile([128, 8, 6], F32, tag="E2")
                nc.vector.tensor_tensor(out=E2[:], in0=E_sl[:], in1=t_lr[:],
                                        op=ALU.max)
                EX = e_pool.tile([128, 8, 6], F32, tag="EX")
                nc.scalar.activation(EX[:], E2[:], AF.Exp)

                pdn = ps_sm.tile([128, 48], F32, tag="pbc")
                nc.tensor.matmul(pdn[:], p0_sb[:, 128 * s:128 * (s + 1)],
                                 EX[:].rearrange("p b h -> p (b h)"),
                                 start=True, stop=True)
                dple = e_pool.tile([128, 48], F32, tag="dple")
                nc.vector.tensor_scalar_add(dple[:], pdn[:], 1e-16)
                rd = e_pool.tile([128, 48], F32, tag="rd")
                nc.vector.reciprocal(rd[:], dple[:])
                rdm = e_pool.tile([128, 48], F32, tag="rdm")
                nc.vector.tensor_mul(rdm[:], rd[:], diagmask_sb[:])
                prd = ps_sm.tile([128, 48], F32, tag="pbc")
                nc.tensor.matmul(prd[:], p0stk_sb[:, 128 * s:128 * (s + 1)],
                                 rdm[:], start=True, stop=True)
                A_sl = e_pool.tile([128, 8, 6], BF16, tag="A_sl")
                nc.vector.tensor_mul(A_sl[:], EX[:],
                                     prd[:].rearrange("p (b h) -> p b h", b=8))

                M_sl = m_pool.tile([128, 8, 96], BF16, tag="M_sl")
                nc.vector.tensor_mul(
                    M_sl[:].rearrange("p b (h d) -> p b h d", h=6),
                    p0rep_sb[:, 96 * 8 * s:96 * 8 * (s + 1)]
                    .rearrange("p (b h d) -> p b h d", b=8, h=6),
                    A_sl[:].unsqueeze(3).broadcast_to([128, 8, 6, 16]))

                # ---- scatter: Gt cols [ds][h*128 + b*16 + dl] ----
                Gt = gt_pool.tile([128, kt, 768], BF16, tag="Gt")
                for b in range(8):
                    pgt = ps_gt.tile([128, kt * 96], F32, tag="pgt")
                    for ds in range(kt):
                        nc.tensor.matmul(pgt[:, 96 * ds:96 * (ds + 1)],
                                         Fg[:, b, 128 * ds:128 * (ds + 1)],
                                         M_sl[:, b, :], start=True, stop=True)
                    eng_copy = (nc.vector.tensor_copy if b % 2 == 0
                                else nc.scalar.copy)
                    eng_copy(
                        Gt[:].rearrange("p d (h2 b2 e) -> p d h2 b2 e",
                                        h2=6, b2=8)[:, :, :, b, :],
                        pgt[:].rearrange("p (d h2 e) -> p d h2 e", d=kt, h2=6))

                # ---- feature matmul (bias row first) ----
                psum_f = ps_f.tile([128, CP], F32, tag="psum_f")
                nmm = kt * 6
                i_mm = 0
                if g < 4:
                    nc.tensor.matmul(psum_f[:], onesrow_sb[:], biasrow_sb[g][:],
                                     start=True, stop=False)
                for ds in range(kt):
                    for h in range(6):
                        nc.tensor.matmul(psum_f[:], Gt[:, ds, 128 * h:128 * (h + 1)],
                                         wp_t[:, (ds * 6 + h) * CP:(ds * 6 + h + 1) * CP],
                                         start=(g == 4 and i_mm == 0),
                                         stop=(i_mm == nmm - 1))
                        i_mm += 1

                # ---- postprocess ----
                if stage < 5:
                    produce(stage, s, psum_f)
                else:
                    cnode = CSTK[:, s, 2:4]
                    tcf = small.tile([128, 2], F32, tag="coord_t")
                    nc.vector.tensor_scalar(tcf[:], cnode, cfac_sb[:, s, :], None,
                                            mybir.AluOpType.mult)
                    nc.vector.tensor_add(OUTC[:, s, :], psum_f[:, 0:2], tcf[:])

                if GATHER_MODE == "hoisted" and 2 <= s <= 7:
                    emit_indirect(stage, s + 2)
                elif GATHER_MODE == "indirect" and s <= 7:
                    emit_indirect(stage, s + 2)

            if stage < 5:
                sdrep_build()

        nc.sync.dma_start(out_t[:].rearrange("(s p) c -> p s c", p=128), OUTC[:])

    nc.compile()
    return nc


# ================================================================
def _host_prep(inputs):
    data = np.asarray(inputs["data"], np.float32)
    eidx = np.asarray(inputs["edge_idx"])
    src_a, dst_a = eidx[0].astype(np.int64), eidx[1].astype(np.int64)
    order = np.argsort(dst_a, kind="stable")
    src_s, dst_s = src_a[order], dst_a[order]
    indeg = np.bincount(dst_a, minlength=N)

    shared = {}
    linW = np.zeros((16, 254), np.float32)
    linW[0:10] = np.asarray(inputs["lin_W"], np.float32)
    shared["linW"] = linW.astype(BF)
    shared["bias1row"] = np.asarray(inputs["lin_b"], np.float32)[None, :].astype(BF)
    shared["onesrow"] = np.ones((1, 128), BF)
    shared["ident"] = np.eye(128, dtype=BF)
    dmask = np.zeros((128, 48), np.float32)
    for b in range(8):
        dmask[16 * b:16 * (b + 1), 6 * b:6 * (b + 1)] = 1.0
    shared["diagmask"] = dmask
    for i in range(1, 5):
        din, C = GAT_DIMS[i - 1]
        kt = din // 128
        CP = 2 if i == 4 else C + 2
        W = np.asarray(inputs[f"W{i}"], np.float32).reshape(din, H, C)
        wp = np.zeros((din, H, CP), np.float32)
        if i < 4:
            wp[:, :, :C] = W / H
            brow = np.zeros((1, CP), np.float32)
            brow[0, :C] = np.asarray(inputs[f"b{i}"], np.float32)
            shared[f"biasrow{i}"] = brow.astype(BF)
        wp[0, :, CP - 2] = 1.0 / H
        wp[1, :, CP - 1] = 1.0 / H
        wp_h = np.zeros((128, kt * H * CP), np.float32)
        for ds in range(kt):
            wp_h[:, ds * H * CP:(ds + 1) * H * CP] = \
                wp[ds * 128:(ds + 1) * 128].reshape(128, H * CP)
        shared[f"wp{i}"] = wp_h.astype(BF)
        a_s = np.asarray(inputs[f"as{i}"], np.float32)
        a_d = np.asarray(inputs[f"ad{i}"], np.float32)
        ws = np.einsum("dhc,hc->dh", W, a_s)
        wd = np.einsum("dhc,hc->dh", W, a_d)
        wsc = np.concatenate([ws, wd], 1)
        ktf = FDIM[i - 1] // 128
        wsc_h = np.zeros((128, ktf * 12), np.float32)
        for ds in range(ktf):
            wsc_h[:, ds * 12:(ds + 1) * 12] = wsc[ds * 128:(ds + 1) * 128]
        shared[f"wsc{i}"] = wsc_h.astype(BF)

    in_maps = []
    for r in range(NCORES):
        m = dict(shared)
        lo, hi = NL * r, NL * (r + 1)
        dT = np.zeros((16, NLP), np.float32)
        dT[0:10, 0:NL] = data[lo:hi].T
        m["dataT"] = dT.astype(BF)
        cl = np.zeros((NLP, 2), np.float32)
        cl[0:NL] = data[lo:hi, 0:2]
        m["coords_loc"] = cl
        cf = np.ones((NLP, 1), np.float32)
        cf[0:NL, 0] = (indeg[lo:hi] == 0).astype(np.float32)
        m["cfac"] = cf

        sel = (dst_s >= lo) & (dst_s < hi)
        es, ed = src_s[sel], dst_s[sel] - lo
        p0 = np.zeros((128, NBLK * 16), np.float32)
        p0rep = np.zeros((128, NBLK * 96), np.float32)
        sidx = np.zeros((128, NBLK * 8), np.int16)
        sidx32 = np.zeros((128, NSLAB * 8), np.int32)
        blk = ed // 16
        for c in range(NBLK):
            emask = blk == c
            k = int(emask.sum())
            assert k <= CAP, f"block overflow core {r} blk {c}: {k}"
            if k == 0:
                continue
            srcs = es[emask]
            lds = ed[emask].astype(np.int64)
            dls = lds % 16
            p0c = np.zeros((128, 16), np.float32)
            p0c[np.arange(k), dls] = 1.0
            p0[:, 16 * c:16 * (c + 1)] = p0c
            p0rep[:, 96 * c:96 * (c + 1)] = np.tile(p0c, (1, 6))
            rr = srcs // NL
            ii = srcs % NL
            # feat table row for node (rr, ii)
            if AG_MODE == "chunked":
                ch = ii // CHROWS
                agrow = ch * (NCORES * CHROWS) + rr * CHROWS + (ii - ch * CHROWS)
            else:
                agrow = rr * NLP + ii
            fulls = np.zeros(128, np.int64)
            fulls[:k] = agrow
            s_i, b_i = c // 8, c % 8
            sidx32[:, 8 * s_i + b_i] = fulls
            ws_ = sidx[:, 64 * s_i:64 * (s_i + 1)]
            for e_i in range(128):
                gk = 128 * b_i + e_i
                ws_[gk % 16, gk // 16] = fulls[e_i]
        for s_i in range(NSLAB):
            w = sidx[:, 64 * s_i:64 * (s_i + 1)]
            w[16:] = np.tile(w[:16], (7, 1))
        # per-slab transposed one-hot for pbc/prd stationary
        p0stk = np.zeros((128, NSLAB * 128), np.float32)
        for s_i in range(NSLAB):
            p0stk[:, 128 * s_i:128 * (s_i + 1)] = \
                p0[:, 128 * s_i:128 * (s_i + 1)].T
        m["p0"] = p0
        m["p0stk"] = p0stk
        m["p0rep"] = p0rep.astype(BF)
        m["srcidx"] = sidx
        m["sidx32"] = sidx32
        in_maps.append(m)
    return in_maps


_NC_CACHE = None


def kernel(**inputs):
    global _NC_CACHE
    in_maps = _host_prep(inputs)
    if _NC_CACHE is None:
        _NC_CACHE = _build_nc()
    res = run_bass_kernel_spmd(_NC_CACHE, in_maps, core_ids=list(range(NCORES)))
    out = np.zeros((N, 2), np.float32)
    for r in range(NCORES):
        out[NL * r:NL * (r + 1)] = res.results[r]["out"][:NL]
    return out
